# revision 1
# baseline (speedup 1.0000x reference)
# HEPOS cross-attention (strided per-head K/V) on 8 Trainium2 NeuronCores.
#
# Reference computation (per head h, stride s = STRIDE_LIST[h]):
#   Q = x @ Wq.T + bq ; K = e @ Wk.T + bk ; V = e @ Wv.T + bv
#   out_h = softmax(Q_h @ K_h[::s].T / 8) @ V_h[::s]
#   out   = concat_h(out_h) @ Wo.T + bo
#
# Sharding: 64 (batch, head) units over 8 cores. Core c owns head group
# g = c % 4 (heads 4g..4g+3, strides [1,2,4,8] -- one of each stride, so
# per-core work is identical) and batch pair [0,1] (c < 4) or [2,3]
# (c >= 4). Each core computes its heads' contribution to out (partial
# out = concat @ Wo cols) for its two batches; the host sums the four
# partials per batch and adds bo.
#
# On-device layout convention: activations live transposed (D on the
# SBUF partition dim), prepared by the host with numpy. The stride is
# folded into the K/V projections (only strided encoder rows are
# projected). Scores are computed transposed ([S_chunk, T]) so the
# attention matmul needs no transposes anywhere; the softmax denominator
# falls out of a ones-column appended to the V stationary operand.

import os
import sys

import ml_dtypes
import numpy as np

BF16 = ml_dtypes.bfloat16

for _p in ("/opt/trn_rl_repo", "/root/.axon_site/_ro/trn_rl_repo"):
    if os.path.isdir(_p) and _p not in sys.path:
        sys.path.insert(0, _p)

import concourse.bass as bass  # noqa: E402
import concourse.tile as tile  # noqa: E402
from concourse import bacc, mybir  # noqa: E402
from concourse import bass_utils  # noqa: E402

F32 = mybir.dt.float32
MM_DT = mybir.dt.bfloat16  # matmul operand dtype: full PE rate, half DMA
AF = mybir.ActivationFunctionType

D_MODEL = 1024
NUM_HEADS = 16
HEAD_DIM = 64
STRIDE_LIST = [1, 2, 4, 8] * 4
B, T, S = 4, 1024, 4096
N_CORES = 8

FULL_CFG = dict(
    nb=2,  # batches per core
    T=T,
    S=S,
    D=D_MODEL,
    nh=4,  # heads per core
    strides=(1, 2, 4, 8),
    hd=HEAD_DIM,
    blk=512,  # encoder S-block columns processed per iteration
    tt=512,  # T tile (PSUM free-dim limit for fp32)
)


FR = MM_DT  # fp32r: all matmul operands are produced/stored in this dtype


def _mm(nc, out, lhsT, rhs, start, stop):
    nc.tensor.matmul(out, lhsT, rhs, start=start, stop=stop)


def build_program(cfg):
    """Build the per-core Bass/Tile program (same program on all cores)."""
    nb, Tl, Sl, Dl = cfg["nb"], cfg["T"], cfg["S"], cfg["D"]
    nh, strides, hd = cfg["nh"], cfg["strides"], cfg["hd"]
    blk, tt = cfg["blk"], cfg["tt"]
    ndc = Dl // 128  # contraction chunks
    nblk = Sl // blk
    ntt = Tl // tt
    nhp = (nh * hd + 127) // 128  # 128-row passes over the packed heads
    HP = nh * hd  # packed head rows (e.g. 256)

    nc = bacc.Bacc(
        "TRN2",
        target_bir_lowering=False,
        debug=False,
        enable_asserts=False,
        num_devices=N_CORES,
    )

    xT = nc.dram_tensor("xT", [Dl, nb * Tl], MM_DT, kind="ExternalInput").ap()
    eT = nc.dram_tensor("eT", [Dl, nb * Sl], MM_DT, kind="ExternalInput").ap()
    wqT = nc.dram_tensor("wqT", [Dl, HP], MM_DT, kind="ExternalInput").ap()
    wkT = nc.dram_tensor("wkT", [Dl, HP], MM_DT, kind="ExternalInput").ap()
    wvT = nc.dram_tensor("wvT", [Dl, HP], MM_DT, kind="ExternalInput").ap()
    woT = nc.dram_tensor("woT", [HP, Dl], MM_DT, kind="ExternalInput").ap()
    bqd = nc.dram_tensor("bq", [nh, hd, 1], F32, kind="ExternalInput").ap()
    bkd = nc.dram_tensor("bk", [nh, hd, 1], F32, kind="ExternalInput").ap()
    bvd = nc.dram_tensor("bvb", [128, HP], F32, kind="ExternalInput").ap()
    out = nc.dram_tensor("partial", [nb * Tl, Dl], F32, kind="ExternalOutput").ap()

    with tile.TileContext(nc) as tc:
        _build_tile(tc, cfg, xT, eT, wqT, wkT, wvT, woT, bqd, bkd, bvd, out)

    nc.compile()
    return nc


def _build_tile(tc, cfg, xT, eT, wqT, wkT, wvT, woT, bqd, bkd, bvd, out):
    nc = tc.nc
    nb, Tl, Sl, Dl = cfg["nb"], cfg["T"], cfg["S"], cfg["D"]
    nh, strides, hd = cfg["nh"], cfg["strides"], cfg["hd"]
    blk, tt = cfg["blk"], cfg["tt"]
    ndc = Dl // 128
    nblk = Sl // blk
    ntt = Tl // tt
    HP = nh * hd
    nhp = (HP + 127) // 128

    from contextlib import ExitStack

    with ExitStack() as ctx:
        wpool = ctx.enter_context(tc.tile_pool(name="weights", bufs=1))
        qtpool = ctx.enter_context(tc.tile_pool(name="qt", bufs=1))
        etpool = ctx.enter_context(tc.tile_pool(name="et", bufs=2))
        ktpool = ctx.enter_context(tc.tile_pool(name="kt", bufs=3))
        vpool = ctx.enter_context(tc.tile_pool(name="v", bufs=8))
        ppool = ctx.enter_context(tc.tile_pool(name="p", bufs=3))
        avpool = ctx.enter_context(tc.tile_pool(name="avacc", bufs=1))
        opool = ctx.enter_context(tc.tile_pool(name="outs", bufs=3))
        npool = ctx.enter_context(tc.tile_pool(name="norm", bufs=1))
        qo_ps = ctx.enter_context(tc.tile_pool(name="qo_ps", bufs=2, space="PSUM"))
        sc_ps = ctx.enter_context(tc.tile_pool(name="sc_ps", bufs=2, space="PSUM"))
        kv_ps = ctx.enter_context(tc.tile_pool(name="kv_ps", bufs=2, space="PSUM"))
        av_ps = ctx.enter_context(tc.tile_pool(name="av_ps", bufs=2, space="PSUM"))

        # ---- weights into SBUF ----
        wq_sb = [wpool.tile([128, HP], FR, tag=f"wq{dc}", name="wq_sb") for dc in range(ndc)]
        wk_sb = [wpool.tile([128, HP], FR, tag=f"wk{dc}", name="wk_sb") for dc in range(ndc)]
        wv_sb = [wpool.tile([128, HP], FR, tag=f"wv{dc}", name="wv_sb") for dc in range(ndc)]
        for dc in range(ndc):
            nc.sync.dma_start(out=wq_sb[dc], in_=wqT[dc * 128 : (dc + 1) * 128, :])
            nc.sync.dma_start(out=wk_sb[dc], in_=wkT[dc * 128 : (dc + 1) * 128, :])
            nc.sync.dma_start(out=wv_sb[dc], in_=wvT[dc * 128 : (dc + 1) * 128, :])
        wo_sb = [wpool.tile([hd, Dl], FR, tag=f"wo{h}", name="wo_sb") for h in range(nh)]
        for h in range(nh):
            nc.sync.dma_start(out=wo_sb[h], in_=woT[h * hd : (h + 1) * hd, :])
        bq_sb = wpool.tile([hd, nh], F32, tag="bq", name="bq_sb")
        bk_sb = wpool.tile([hd, nh], F32, tag="bk", name="bk_sb")
        for h in range(nh):
            nc.sync.dma_start(out=bq_sb[:, h : h + 1], in_=bqd[h])
            nc.sync.dma_start(out=bk_sb[:, h : h + 1], in_=bkd[h])
        bv_sb = wpool.tile([128, HP], F32, tag="bv", name="bv_sb")
        nc.sync.dma_start(out=bv_sb, in_=bvd)
        ones_sb = wpool.tile([128, 1], F32, tag="ones", name="ones_sb")
        nc.vector.memset(ones_sb, 1.0)

        # ---- phase 1: Q^T = (x @ Wq.T + bq)^T, packed heads on partitions ----
        qt_sb = {}  # (b, pass) -> [128, T] tile
        with tc.tile_pool(name="xt", bufs=1) as xpool:
            for b in range(nb):
                xts = []
                for dc in range(ndc):
                    xt = xpool.tile([128, Tl], FR, tag=f"xt{dc}", name="xt")
                    nc.sync.dma_start(
                        out=xt,
                        in_=xT[dc * 128 : (dc + 1) * 128, b * Tl : (b + 1) * Tl],
                    )
                    xts.append(xt)
                for h in range(nh):
                    qt = qtpool.tile([hd, Tl], FR, tag=f"qt{b}{h}", name="qt")
                    qt_sb[(b, h)] = qt
                    for nt in range(ntt):
                        ps = qo_ps.tile([128, tt], F32, tag="qo", name="q_psum")
                        for dc in range(ndc):
                            _mm(
                                nc,
                                ps[:hd, :],
                                wq_sb[dc][:, h * hd : (h + 1) * hd],
                                xts[dc][:, nt * tt : (nt + 1) * tt],
                                start=(dc == 0),
                                stop=(dc == ndc - 1),
                            )
                        nc.scalar.activation(
                            qt[:, nt * tt : (nt + 1) * tt],
                            ps[:hd, :],
                            AF.Identity,
                            bias=bq_sb[:, h : h + 1],
                        )

        # ---- phase 2+3 per batch ----
        for b in range(nb):
            avacc = {}
            for h in range(nh):
                avacc[h] = avpool.tile([hd + 1, Tl], F32, tag=f"av{h}", name="avacc")
            for ib in range(nblk):
                et = []
                for dc in range(ndc):
                    t = etpool.tile([128, blk], FR, tag=f"et{dc}", name="et_t")
                    nc.sync.dma_start(
                        out=t,
                        in_=eT[
                            dc * 128 : (dc + 1) * 128,
                            b * Sl + ib * blk : b * Sl + (ib + 1) * blk,
                        ],
                    )
                    et.append(t)
                for h in range(nh):
                    s = strides[h]
                    ncol = blk // s  # strided K/V rows in this block
                    # K^T_h for this block: [hd, ncol]
                    kt = ktpool.tile([hd, blk], FR, name="kt")
                    for c0 in range(0, ncol, tt):
                        cw = min(tt, ncol - c0)
                        kps = kv_ps.tile([128, tt], F32, tag="kv", name="kv_psum")
                        for dc in range(ndc):
                            _mm(
                                nc,
                                kps[:hd, :cw],
                                wk_sb[dc][:, h * hd : (h + 1) * hd],
                                et[dc][:, c0 * s : (c0 + cw) * s : s],
                                start=(dc == 0),
                                stop=(dc == ndc - 1),
                            )
                        nc.scalar.activation(
                            kt[:, c0 : c0 + cw],
                            kps[:hd, :cw],
                            AF.Identity,
                            bias=bk_sb[:, h : h + 1],
                        )
                    # V chunks + scores + exp + AV accumulation
                    nck = (ncol + 127) // 128
                    avp = [
                        av_ps.tile([hd + 1, tt], F32, tag="av_ps", name="av_psum") for _ in range(ntt)
                    ]
                    for ck in range(nck):
                        rw = min(128, ncol - ck * 128)
                        vt = vpool.tile([128, hd + 8], FR, tag="v", name="vt")
                        vps = kv_ps.tile([128, tt], F32, tag="kv", name="kv_psum")
                        for dc in range(ndc):
                            _mm(
                                nc,
                                vps[:rw, :hd],
                                et[dc][:, ck * 128 * s : (ck * 128 + rw) * s : s],
                                wv_sb[dc][:, h * hd : (h + 1) * hd],
                                start=(dc == 0),
                                stop=(dc == ndc - 1),
                            )
                        nc.vector.tensor_add(
                            vt[:rw, :hd], vps[:rw, :hd], bv_sb[:rw, h * hd : (h + 1) * hd]
                        )
                        nc.vector.tensor_copy(vt[:rw, hd : hd + 1], ones_sb[:rw])
                        # scores^T chunk: [rw, T], then P = exp(scores/8)
                        pt = ppool.tile([128, Tl], FR, tag="p", name="pt")
                        for nt in range(ntt):
                            sps = sc_ps.tile([128, tt], F32, tag="sc", name="sc_psum")
                            _mm(
                                nc,
                                sps[:rw, :],
                                kt[:, ck * 128 : ck * 128 + rw],
                                qt_sb[(b, h)][:, nt * tt : (nt + 1) * tt],
                                start=True,
                                stop=True,
                            )
                            nc.scalar.activation(
                                pt[:rw, nt * tt : (nt + 1) * tt],
                                sps[:rw, :],
                                AF.Exp,
                                scale=1.0 / float(np.sqrt(hd)),
                            )
                            _mm(
                                nc,
                                avp[nt],
                                vt[:rw, : hd + 1],
                                pt[:rw, nt * tt : (nt + 1) * tt],
                                start=(ck == 0),
                                stop=(ck == nck - 1),
                            )
                    for nt in range(ntt):
                        dst = avacc[h][:, nt * tt : (nt + 1) * tt]
                        if ib == 0:
                            nc.vector.tensor_copy(dst, avp[nt])
                        else:
                            nc.vector.tensor_add(dst, dst, avp[nt])

            # ---- phase 3: normalize + output projection ----
            ot_sb = {}
            for h in range(nh):
                r = npool.tile([1, Tl], F32, tag="recip", name="recip")
                nc.vector.reciprocal(r, avacc[h][hd : hd + 1, :])
                rb = npool.tile([hd, Tl], F32, tag="rb", name="rbcast")
                nc.gpsimd.partition_broadcast(rb, r)
                ot = npool.tile([hd, Tl], FR, tag=f"ot{h}", name="ot")
                nc.vector.tensor_mul(ot, avacc[h][:hd, :], rb)
                ot_sb[h] = ot
            for tc_i in range(Tl // 128):
                for nt in range(0, Dl, tt):
                    ops = qo_ps.tile([128, tt], F32, tag="qo", name="q_psum")
                    for h in range(nh):
                        _mm(
                            nc,
                            ops,
                            ot_sb[h][:, tc_i * 128 : (tc_i + 1) * 128],
                            wo_sb[h][:, nt : nt + tt],
                            start=(h == 0),
                            stop=(h == nh - 1),
                        )
                    ob = opool.tile([128, tt], F32, tag="ob", name="ob")
                    nc.vector.tensor_copy(ob, ops)
                    nc.sync.dma_start(
                        out=out[
                            b * Tl + tc_i * 128 : b * Tl + (tc_i + 1) * 128,
                            nt : nt + tt,
                        ],
                        in_=ob,
                    )


# ---------------------------------------------------------------------------
# Host-side sharding / gathering
# ---------------------------------------------------------------------------


def _core_map():
    """core -> (batches, heads)"""
    m = {}
    for c in range(N_CORES):
        g = c % 4
        bs = [0, 1] if c < 4 else [2, 3]
        hs = [4 * g + i for i in range(4)]
        m[c] = (bs, hs)
    return m


def shard_inputs(inputs, cfg):
    x = np.asarray(inputs["decoder_input"], np.float32)
    e = np.asarray(inputs["encoder_output"], np.float32)
    Wq = np.asarray(inputs["Wq"], np.float32)
    Wk = np.asarray(inputs["Wk"], np.float32)
    Wv = np.asarray(inputs["Wv"], np.float32)
    Wo = np.asarray(inputs["Wo"], np.float32)
    bq = np.asarray(inputs["bq"], np.float32)
    bk = np.asarray(inputs["bk"], np.float32)
    bv = np.asarray(inputs["bv"], np.float32)
    hd = cfg["hd"]
    nh = cfg["nh"]
    in_maps = []
    for c, (bs, hs) in _core_map().items():
        rows = np.concatenate([np.arange(h * hd, (h + 1) * hd) for h in hs])
        xT = np.ascontiguousarray(
            x[bs].reshape(len(bs) * cfg["T"], cfg["D"]).T.astype(BF16)
        )
        eTc = np.ascontiguousarray(
            e[bs].reshape(len(bs) * cfg["S"], cfg["D"]).T.astype(BF16)
        )
        in_maps.append(
            {
                "xT": xT,
                "eT": eTc,
                "wqT": np.ascontiguousarray(Wq[rows].T.astype(BF16)),
                "wkT": np.ascontiguousarray(Wk[rows].T.astype(BF16)),
                "wvT": np.ascontiguousarray(Wv[rows].T.astype(BF16)),
                "woT": np.ascontiguousarray(Wo[:, rows].T.astype(BF16)),
                "bq": np.ascontiguousarray(bq[rows].reshape(nh, hd, 1)),
                "bk": np.ascontiguousarray(bk[rows].reshape(nh, hd, 1)),
                "bvb": np.ascontiguousarray(
                    np.tile(bv[rows][None, :], (128, 1))
                ),
            }
        )
    return in_maps


def gather_output(results, bo, cfg):
    Tl, Dl = cfg["T"], cfg["D"]
    out = np.zeros((B, Tl, Dl), np.float32)
    for c, (bs, _hs) in _core_map().items():
        p = results[c]["partial"].reshape(len(bs), Tl, Dl)
        for i, b in enumerate(bs):
            out[b] += p[i]
    return out + np.asarray(bo, np.float32)[None, None, :]


_COMPILED = None


def _get_compiled():
    global _COMPILED
    if _COMPILED is None:
        _COMPILED = build_program(FULL_CFG)
    return _COMPILED


def run_on_cores(inputs, trace=False, **kw):
    nc = _get_compiled()
    in_maps = shard_inputs(inputs, FULL_CFG)
    res = bass_utils.run_bass_kernel_spmd(
        nc, in_maps, core_ids=list(range(N_CORES)), trace=trace, **kw
    )
    return res


def kernel(**inputs) -> np.ndarray:
    res = run_on_cores(inputs, trace=False)
    return gather_output(res.results, inputs["bo"], FULL_CFG)



# revision 2
# speedup vs baseline: 1.1542x; 1.1542x over previous
# HEPOS cross-attention (strided per-head K/V) on 8 Trainium2 NeuronCores.
#
# Reference computation (per head h, stride s = STRIDE_LIST[h]):
#   Q = x @ Wq.T + bq ; K = e @ Wk.T + bk ; V = e @ Wv.T + bv
#   out_h = softmax(Q_h @ K_h[::s].T / 8) @ V_h[::s]
#   out   = concat_h(out_h) @ Wo.T + bo
#
# Sharding: 64 (batch, head) units over 8 cores. Core c owns head group
# g = c % 4 (heads 4g..4g+3, strides [1,2,4,8]) and batch pair [0,1]
# (c < 4) or [2,3] (c >= 4). Each core computes its heads' contribution
# to out; the host sums the four partials per batch and adds bo.
#
# On-device design (v2):
#  * Heads are processed as two stride PAIRS (sA, 2*sA): (1,2) and (4,8).
#    Head A of a pair lives on SBUF partitions 0-63, head B on 64-127.
#  * Q/K/V projections use the weight matrix as the matmul stationary with
#    both heads packed into the 128 stationary columns (full PE width).
#    K^T/V^T stream the "union" encoder columns (stride sA); head B rows
#    are valid at even union columns and are compacted on evacuation.
#    The stride-4 union for pair (4,8) is pre-packed by the host (eT4) so
#    all DMA stays contiguous.
#  * V^T ([hd, keys]) is flipped to AV orientation ([keys, hd]) with DMA
#    XBAR transposes (2-byte dtype, 16x128 tiles) - zero PE cost.
#  * Scores are computed transposed ([keys, T]); head B's score matmuls
#    use partitions 64-127 (PE row-tile T8) and overlap head A's (T0).
#  * AV accumulates into PSUM tiles that stay resident across all encoder
#    blocks of a (batch, pair); the softmax denominator falls out of a
#    ones-column appended to the V stationary.
#  * Scores of chunk k+1 are issued before AV of chunk k so the PE never
#    waits on the scalar engine's exp.

import os
import sys

import ml_dtypes
import numpy as np

BF16 = ml_dtypes.bfloat16

for _p in ("/opt/trn_rl_repo", "/root/.axon_site/_ro/trn_rl_repo"):
    if os.path.isdir(_p) and _p not in sys.path:
        sys.path.insert(0, _p)

import concourse.bass as bass  # noqa: E402
import concourse.tile as tile  # noqa: E402
from concourse import bacc, mybir  # noqa: E402
from concourse import bass_utils  # noqa: E402

F32 = mybir.dt.float32
MM_DT = mybir.dt.bfloat16  # matmul operand dtype: full PE rate, half DMA
AF = mybir.ActivationFunctionType

D_MODEL = 1024
NUM_HEADS = 16
HEAD_DIM = 64
STRIDE_LIST = [1, 2, 4, 8] * 4
B, T, S = 4, 1024, 4096
N_CORES = 8

FULL_CFG = dict(
    nb=2,  # batches per core
    T=T,
    S=S,
    D=D_MODEL,
    strides=(1, 2, 4, 8),  # per-core head strides; pairs (s0,s1),(s2,s3)
    hd=HEAD_DIM,
    blk=1024,  # encoder S-block (stride-1 columns) per iteration
    tt=512,  # T tile (PSUM free-dim limit for fp32)
)

FR = MM_DT


def _mm(nc, out, lhsT, rhs, start, stop):
    nc.tensor.matmul(out, lhsT, rhs, start=start, stop=stop)


def build_program(cfg):
    """Build the per-core Bass/Tile program (same program on all cores)."""
    nb, Tl, Sl, Dl = cfg["nb"], cfg["T"], cfg["S"], cfg["D"]
    strides, hd = cfg["strides"], cfg["hd"]
    assert strides[1] == 2 * strides[0] and strides[3] == 2 * strides[2]
    HP = 4 * hd  # packed head rows (256)
    s4 = strides[2]

    nc = bacc.Bacc(
        "TRN2",
        target_bir_lowering=False,
        debug=False,
        enable_asserts=False,
        num_devices=N_CORES,
    )

    xT = nc.dram_tensor("xT", [Dl, nb * Tl], MM_DT, kind="ExternalInput").ap()
    eT = nc.dram_tensor("eT", [Dl, nb * Sl], MM_DT, kind="ExternalInput").ap()
    eT4 = nc.dram_tensor(
        "eT4", [Dl, nb * (Sl // s4)], MM_DT, kind="ExternalInput"
    ).ap()
    wqT = nc.dram_tensor("wqT", [Dl, HP], MM_DT, kind="ExternalInput").ap()
    wkT = nc.dram_tensor("wkT", [Dl, HP], MM_DT, kind="ExternalInput").ap()
    wvT = nc.dram_tensor("wvT", [Dl, HP], MM_DT, kind="ExternalInput").ap()
    woT = nc.dram_tensor("woT", [HP, Dl], MM_DT, kind="ExternalInput").ap()
    bqd = nc.dram_tensor("bq", [2, 128, 1], F32, kind="ExternalInput").ap()
    bkd = nc.dram_tensor("bk", [2, 128, 1], F32, kind="ExternalInput").ap()
    bvd = nc.dram_tensor("bv", [2, 128, 1], F32, kind="ExternalInput").ap()
    out = nc.dram_tensor("partial", [nb * Tl, Dl], F32, kind="ExternalOutput").ap()

    with tile.TileContext(nc) as tc:
        _build_tile(tc, cfg, xT, eT, eT4, wqT, wkT, wvT, woT, bqd, bkd, bvd, out)

    nc.compile()
    return nc


def _build_tile(tc, cfg, xT, eT, eT4, wqT, wkT, wvT, woT, bqd, bkd, bvd, out):
    nc = tc.nc
    nb, Tl, Sl, Dl = cfg["nb"], cfg["T"], cfg["S"], cfg["D"]
    strides, hd = cfg["strides"], cfg["hd"]
    blk, tt = cfg["blk"], cfg["tt"]
    ndc = Dl // 128
    nblk = Sl // blk
    ntt = Tl // tt
    HP = 4 * hd
    scale = 1.0 / float(np.sqrt(hd))

    from contextlib import ExitStack

    with ExitStack() as ctx:
        wpool = ctx.enter_context(tc.tile_pool(name="weights", bufs=1))
        qtpool = ctx.enter_context(tc.tile_pool(name="qt", bufs=1))
        etpool = ctx.enter_context(tc.tile_pool(name="et", bufs=2))
        ktpool = ctx.enter_context(tc.tile_pool(name="kt", bufs=2))
        vtpool = ctx.enter_context(tc.tile_pool(name="vtT", bufs=2))
        vpool = ctx.enter_context(tc.tile_pool(name="v", bufs=12))
        ppool = ctx.enter_context(tc.tile_pool(name="p", bufs=3))
        npool = ctx.enter_context(tc.tile_pool(name="norm", bufs=2))
        otpool = ctx.enter_context(tc.tile_pool(name="ot", bufs=2))
        obpool = ctx.enter_context(tc.tile_pool(name="outs", bufs=3))
        sc_ps = ctx.enter_context(tc.tile_pool(name="sc_ps", bufs=2, space="PSUM"))
        kv_ps = ctx.enter_context(tc.tile_pool(name="kv_ps", bufs=2, space="PSUM"))
        av_ps = ctx.enter_context(tc.tile_pool(name="av_ps", bufs=1, space="PSUM"))

        # ---- weights into SBUF ----
        wq_sb = [wpool.tile([128, HP], FR, tag=f"wq{dc}", name="wq_sb") for dc in range(ndc)]
        wk_sb = [wpool.tile([128, HP], FR, tag=f"wk{dc}", name="wk_sb") for dc in range(ndc)]
        wv_sb = [wpool.tile([128, HP], FR, tag=f"wv{dc}", name="wv_sb") for dc in range(ndc)]
        for dc in range(ndc):
            nc.sync.dma_start(out=wq_sb[dc], in_=wqT[dc * 128 : (dc + 1) * 128, :])
            nc.sync.dma_start(out=wk_sb[dc], in_=wkT[dc * 128 : (dc + 1) * 128, :])
            nc.sync.dma_start(out=wv_sb[dc], in_=wvT[dc * 128 : (dc + 1) * 128, :])
        wo_sb = [wpool.tile([128, Dl], FR, tag=f"wo{p}", name="wo_sb") for p in range(2)]
        for p in range(2):
            nc.sync.dma_start(out=wo_sb[p], in_=woT[p * 128 : (p + 1) * 128, :])
        bq_sb = wpool.tile([128, 2], F32, tag="bq", name="bq_sb")
        bk_sb = wpool.tile([128, 2], F32, tag="bk", name="bk_sb")
        bv_sb = wpool.tile([128, 2], F32, tag="bv", name="bv_sb")
        for p in range(2):
            nc.sync.dma_start(out=bq_sb[:, p : p + 1], in_=bqd[p])
            nc.sync.dma_start(out=bk_sb[:, p : p + 1], in_=bkd[p])
            nc.sync.dma_start(out=bv_sb[:, p : p + 1], in_=bvd[p])
        ones_sb = wpool.tile([128, 1], F32, tag="ones", name="ones_sb")
        nc.vector.memset(ones_sb, 1.0)

        # ---- phase 1: Q^T = (x @ Wq.T + bq)^T, head pairs on partitions ----
        qt_sb = {}  # (b, pair) -> [128, T] tile
        with tc.tile_pool(name="xt", bufs=1) as xpool:
            for b in range(nb):
                xts = []
                for dc in range(ndc):
                    xt = xpool.tile([128, Tl], FR, tag=f"xt{dc}", name="xt")
                    nc.sync.dma_start(
                        out=xt,
                        in_=xT[dc * 128 : (dc + 1) * 128, b * Tl : (b + 1) * Tl],
                    )
                    xts.append(xt)
                for p in range(2):
                    qt = qtpool.tile([128, Tl], FR, tag=f"qt{b}{p}", name="qt")
                    qt_sb[(b, p)] = qt
                    for nt in range(ntt):
                        ps = sc_ps.tile([128, tt], F32, tag="sc", name="sc_psum")
                        for dc in range(ndc):
                            _mm(
                                nc,
                                ps,
                                wq_sb[dc][:, p * 128 : (p + 1) * 128],
                                xts[dc][:, nt * tt : (nt + 1) * tt],
                                start=(dc == 0),
                                stop=(dc == ndc - 1),
                            )
                        nc.scalar.activation(
                            qt[:, nt * tt : (nt + 1) * tt],
                            ps,
                            AF.Identity,
                            bias=bq_sb[:, p : p + 1],
                        )

        # ---- phase 2: attention per (batch, pair), out proj per batch ----
        ot_sb = {}
        for b in range(nb):
            for p in range(2):
                sA = strides[2 * p]
                src = eT if p == 0 else eT4
                Scols = Sl // sA  # union columns per batch for this pair
                ublk = blk // sA  # union columns per block
                nA = ublk // 128  # A-key chunks per block
                nB = nA // 2  # B-key chunks per block
                avp = {}
                for h in range(2):
                    for nt in range(ntt):
                        avp[(h, nt)] = av_ps.tile(
                            [hd + 1, tt], F32, tag=f"av{h}{nt}", name="av_psum"
                        )

                pending = []  # AV emissions delayed by one chunk-step

                def flush_pending():
                    for vt, pt, h, first, last in pending:
                        for nt in range(ntt):
                            _mm(
                                nc,
                                avp[(h, nt)],
                                vt[:, : hd + 1],
                                pt[:, nt * tt : (nt + 1) * tt],
                                start=first,
                                stop=last,
                            )
                    pending.clear()

                for ib in range(nblk):
                    et = []
                    for dc in range(ndc):
                        t = etpool.tile([128, ublk], FR, tag=f"et{dc}", name="et_t")
                        nc.sync.dma_start(
                            out=t,
                            in_=src[
                                dc * 128 : (dc + 1) * 128,
                                b * Scols + ib * ublk : b * Scols + (ib + 1) * ublk,
                            ],
                        )
                        et.append(t)

                    # K^T proj (packed pair; B compacted to even union cols)
                    kt_A = ktpool.tile([64, ublk], FR, tag="ktA", name="kt_A")
                    kt_B = ktpool.tile([128, ublk // 2], FR, tag="ktB", name="kt_B")
                    vtT_A = vtpool.tile([64, ublk], FR, tag="vtA", name="vtT_A")
                    vtT_B = vtpool.tile([128, ublk // 2], FR, tag="vtB", name="vtT_B")
                    for c0 in range(0, ublk, tt):
                        cw = min(tt, ublk - c0)
                        kps = kv_ps.tile([128, tt], F32, tag="kv", name="kv_psum")
                        for dc in range(ndc):
                            _mm(
                                nc,
                                kps[:, :cw],
                                wk_sb[dc][:, p * 128 : (p + 1) * 128],
                                et[dc][:, c0 : c0 + cw],
                                start=(dc == 0),
                                stop=(dc == ndc - 1),
                            )
                        nc.vector.tensor_scalar_add(
                            kt_A[:, c0 : c0 + cw], kps[0:64, :cw], bk_sb[0:64, p : p + 1]
                        )
                        nc.vector.tensor_scalar_add(
                            kt_B[64:128, c0 // 2 : (c0 + cw) // 2],
                            kps[64:128, 0:cw:2],
                            bk_sb[64:128, p : p + 1],
                        )
                        # V^T proj (same streaming, wv stationary)
                        vps = kv_ps.tile([128, tt], F32, tag="kv", name="kv_psum")
                        for dc in range(ndc):
                            _mm(
                                nc,
                                vps[:, :cw],
                                wv_sb[dc][:, p * 128 : (p + 1) * 128],
                                et[dc][:, c0 : c0 + cw],
                                start=(dc == 0),
                                stop=(dc == ndc - 1),
                            )
                        nc.vector.tensor_scalar_add(
                            vtT_A[:, c0 : c0 + cw], vps[0:64, :cw], bv_sb[0:64, p : p + 1]
                        )
                        nc.vector.tensor_scalar_add(
                            vtT_B[64:128, c0 // 2 : (c0 + cw) // 2],
                            vps[64:128, 0:cw:2],
                            bv_sb[64:128, p : p + 1],
                        )

                    # V -> [keys, hd] via DMA XBAR transpose (+ ones column)
                    vts_A, vts_B = [], []
                    for ck in range(nA):
                        vt = vpool.tile([128, hd + 1], FR, tag="vA", name="vt_a")
                        nc.vector.tensor_copy(vt[:, hd : hd + 1], ones_sb)
                        nc.sync.dma_start(
                            out=vt[:, 0:hd],
                            in_=vtT_A[0:64, ck * 128 : (ck + 1) * 128],
                            transpose=True,
                        )
                        vts_A.append(vt)
                    for ck in range(nB):
                        vt = vpool.tile([128, hd + 1], FR, tag="vB", name="vt_b", bufs=6)
                        nc.vector.tensor_copy(vt[:, hd : hd + 1], ones_sb)
                        nc.sync.dma_start(
                            out=vt[:, 0:hd],
                            in_=vtT_B[64:128, ck * 128 : (ck + 1) * 128],
                            transpose=True,
                        )
                        vts_B.append(vt)

                    # scores + exp + AV, software-pipelined by one chunk
                    for ck in range(nA):
                        do_B = ck % 2 == 1
                        ckb = ck // 2
                        ptA = ppool.tile([128, Tl], FR, tag="pA", name="ptA")
                        spsA, spsB = [], []
                        if do_B:
                            ptB = ppool.tile([128, Tl], FR, tag="pB", name="ptB")
                        for nt in range(ntt):
                            sa = sc_ps.tile([128, tt], F32, tag="sc", name="sc_psum")
                            _mm(
                                nc,
                                sa,
                                kt_A[:, ck * 128 : (ck + 1) * 128],
                                qt_sb[(b, p)][0:64, nt * tt : (nt + 1) * tt],
                                start=True,
                                stop=True,
                            )
                            spsA.append(sa)
                            if do_B:
                                sb_ = kv_ps.tile([128, tt], F32, tag="kv", name="kv_psum")
                                _mm(
                                    nc,
                                    sb_,
                                    kt_B[64:128, ckb * 128 : (ckb + 1) * 128],
                                    qt_sb[(b, p)][64:128, nt * tt : (nt + 1) * tt],
                                    start=True,
                                    stop=True,
                                )
                                spsB.append(sb_)
                        for nt in range(ntt):
                            nc.scalar.activation(
                                ptA[:, nt * tt : (nt + 1) * tt],
                                spsA[nt],
                                AF.Exp,
                                scale=scale,
                            )
                        if do_B:
                            for nt in range(ntt):
                                nc.scalar.activation(
                                    ptB[:, nt * tt : (nt + 1) * tt],
                                    spsB[nt],
                                    AF.Exp,
                                    scale=scale,
                                )
                        flush_pending()
                        pending.append(
                            (
                                vts_A[ck],
                                ptA,
                                0,
                                ib == 0 and ck == 0,
                                ib == nblk - 1 and ck == nA - 1,
                            )
                        )
                        if do_B:
                            pending.append(
                                (
                                    vts_B[ckb],
                                    ptB,
                                    1,
                                    ib == 0 and ckb == 0,
                                    ib == nblk - 1 and ckb == nB - 1,
                                )
                            )
                flush_pending()

                # ---- normalize: ot rows 0-63 = head A, 64-127 = head B ----
                ot = otpool.tile([128, Tl], FR, tag=f"ot{p}", name="ot")
                ot_sb[(b, p)] = ot
                for h in range(2):
                    for nt in range(ntt):
                        r = npool.tile([1, tt], F32, tag="recip", name="recip")
                        nc.vector.reciprocal(r, avp[(h, nt)][hd : hd + 1, :])
                        rb = npool.tile([hd, tt], F32, tag="rb", name="rbcast")
                        nc.gpsimd.partition_broadcast(rb, r)
                        nc.vector.tensor_mul(
                            ot[h * hd : (h + 1) * hd, nt * tt : (nt + 1) * tt],
                            avp[(h, nt)][0:hd, :],
                            rb,
                        )

            # ---- phase 3: output projection for batch b ----
            for tc_i in range(Tl // 128):
                for nt in range(0, Dl, tt):
                    ops = sc_ps.tile([128, tt], F32, tag="sc", name="sc_psum")
                    for p in range(2):
                        _mm(
                            nc,
                            ops,
                            ot_sb[(b, p)][:, tc_i * 128 : (tc_i + 1) * 128],
                            wo_sb[p][:, nt : nt + tt],
                            start=(p == 0),
                            stop=(p == 1),
                        )
                    ob = obpool.tile([128, tt], F32, tag="ob", name="ob")
                    nc.vector.tensor_copy(ob, ops)
                    nc.sync.dma_start(
                        out=out[
                            b * Tl + tc_i * 128 : b * Tl + (tc_i + 1) * 128,
                            nt : nt + tt,
                        ],
                        in_=ob,
                    )


# ---------------------------------------------------------------------------
# Host-side sharding / gathering
# ---------------------------------------------------------------------------


def _core_map():
    """core -> (batches, heads)"""
    m = {}
    for c in range(N_CORES):
        g = c % 4
        bs = [0, 1] if c < 4 else [2, 3]
        hs = [4 * g + i for i in range(4)]
        m[c] = (bs, hs)
    return m


def shard_inputs(inputs, cfg):
    x = np.asarray(inputs["decoder_input"], np.float32)
    e = np.asarray(inputs["encoder_output"], np.float32)
    Wq = np.asarray(inputs["Wq"], np.float32)
    Wk = np.asarray(inputs["Wk"], np.float32)
    Wv = np.asarray(inputs["Wv"], np.float32)
    Wo = np.asarray(inputs["Wo"], np.float32)
    bq = np.asarray(inputs["bq"], np.float32)
    bk = np.asarray(inputs["bk"], np.float32)
    bv = np.asarray(inputs["bv"], np.float32)
    hd = cfg["hd"]
    s4 = cfg["strides"][2]
    in_maps = []
    for c, (bs, hs) in _core_map().items():
        rows = np.concatenate([np.arange(h * hd, (h + 1) * hd) for h in hs])
        xTc = np.ascontiguousarray(
            x[bs].reshape(len(bs) * cfg["T"], cfg["D"]).T.astype(BF16)
        )
        eTc = np.ascontiguousarray(
            e[bs].reshape(len(bs) * cfg["S"], cfg["D"]).T.astype(BF16)
        )
        e4 = e[bs][:, ::s4, :]  # [nb, S//s4, D]
        eT4c = np.ascontiguousarray(
            e4.reshape(len(bs) * (cfg["S"] // s4), cfg["D"]).T.astype(BF16)
        )
        in_maps.append(
            {
                "xT": xTc,
                "eT": eTc,
                "eT4": eT4c,
                "wqT": np.ascontiguousarray(Wq[rows].T.astype(BF16)),
                "wkT": np.ascontiguousarray(Wk[rows].T.astype(BF16)),
                "wvT": np.ascontiguousarray(Wv[rows].T.astype(BF16)),
                "woT": np.ascontiguousarray(Wo[:, rows].T.astype(BF16)),
                "bq": np.ascontiguousarray(bq[rows].reshape(2, 128, 1)),
                "bk": np.ascontiguousarray(bk[rows].reshape(2, 128, 1)),
                "bv": np.ascontiguousarray(bv[rows].reshape(2, 128, 1)),
            }
        )
    return in_maps


def gather_output(results, bo, cfg):
    Tl, Dl = cfg["T"], cfg["D"]
    out = np.zeros((B, Tl, Dl), np.float32)
    for c, (bs, _hs) in _core_map().items():
        p = results[c]["partial"].reshape(len(bs), Tl, Dl)
        for i, b in enumerate(bs):
            out[b] += p[i]
    return out + np.asarray(bo, np.float32)[None, None, :]


_COMPILED = None


def _get_compiled():
    global _COMPILED
    if _COMPILED is None:
        _COMPILED = build_program(FULL_CFG)
    return _COMPILED


def run_on_cores(inputs, trace=False, **kw):
    nc = _get_compiled()
    in_maps = shard_inputs(inputs, FULL_CFG)
    res = bass_utils.run_bass_kernel_spmd(
        nc, in_maps, core_ids=list(range(N_CORES)), trace=trace, **kw
    )
    return res


def kernel(**inputs) -> np.ndarray:
    res = run_on_cores(inputs, trace=False)
    return gather_output(res.results, inputs["bo"], FULL_CFG)


# revision 9
# speedup vs baseline: 1.2363x; 1.0711x over previous
# HEPOS cross-attention (strided per-head K/V) on 8 Trainium2 NeuronCores.
#
# Reference computation (per head h, stride s = STRIDE_LIST[h]):
#   Q = x @ Wq.T + bq ; K = e @ Wk.T + bk ; V = e @ Wv.T + bv
#   out_h = softmax(Q_h @ K_h[::s].T / 8) @ V_h[::s]
#   out   = concat_h(out_h) @ Wo.T + bo
#
# Sharding: 64 (batch, head) units over 8 cores. Core c owns head group
# g = c % 4 (heads 4g..4g+3, strides [1,2,4,8]) and batch pair [0,1]
# (c < 4) or [2,3] (c >= 4). Each core computes its heads' contribution
# to out; the host sums the four partials per batch and adds bo.
#
# On-device design (v3):
#  * Heads are processed as two stride PAIRS (sA, 2*sA): (1,2) and (4,8).
#    Head A of a pair lives on SBUF partitions 0-63, head B on 64-127.
#  * Q/K/V projections use the weight matrix as the matmul stationary with
#    both heads packed into the 128 stationary columns (full PE width).
#    K^T/V^T stream the "union" encoder columns (stride sA); head B rows
#    are valid at even union columns and are compacted on evacuation.
#    The stride-4 union for pair (4,8) is pre-packed by the host (eT4).
#  * V^T ([hd, keys]) is flipped to AV orientation ([keys, hd]) with ONE
#    DMA XBAR transpose per (block, head) (3D output access pattern) -
#    zero PE cost, one sync-engine dispatch each.
#  * Scores are computed transposed ([keys, T]); head B's score matmuls
#    use partitions 64-127 (PE row-tile T8) and overlap head A's (T0).
#  * AV accumulates into PSUM tiles resident across all encoder blocks of
#    a (batch, pair); the softmax denominator falls out of a ones-column
#    appended to the V stationary.
#  * Scores of chunk k+1 are issued before AV of chunk k so the PE never
#    waits on the scalar engine's exp.
#  * All DRAM->SBUF loads are single merged DMAs ([128, ndc, *] access
#    patterns); PSUM score tiles are bank-pair wide ([128, 2*tt]) so exp
#    and evacuations run as one instruction per tile.

import os
import sys

import ml_dtypes
import numpy as np

BF16 = ml_dtypes.bfloat16

for _p in ("/opt/trn_rl_repo", "/root/.axon_site/_ro/trn_rl_repo"):
    if os.path.isdir(_p) and _p not in sys.path:
        sys.path.insert(0, _p)

import concourse.bass as bass  # noqa: E402
import concourse.tile as tile  # noqa: E402
from concourse import bacc, mybir  # noqa: E402
from concourse import bass_utils  # noqa: E402

F32 = mybir.dt.float32
MM_DT = mybir.dt.bfloat16  # matmul operand dtype: full PE rate, half DMA
AF = mybir.ActivationFunctionType

D_MODEL = 1024
NUM_HEADS = 16
HEAD_DIM = 64
STRIDE_LIST = [1, 2, 4, 8] * 4
B, T, S = 4, 1024, 4096
N_CORES = 8

FULL_CFG = dict(
    nb=2,  # batches per core
    T=T,
    S=S,
    D=D_MODEL,
    strides=(1, 2, 4, 8),  # per-core head strides; pairs (s0,s1),(s2,s3)
    hd=HEAD_DIM,
    blk=1024,  # encoder S-block (stride-1 columns) per iteration
    tt=512,  # T tile (PSUM free-dim limit for fp32)
)

FR = MM_DT


def _mm(nc, out, lhsT, rhs, start, stop):
    nc.tensor.matmul(out, lhsT, rhs, start=start, stop=stop)


def build_program(cfg):
    """Build the per-core Bass/Tile program (same program on all cores)."""
    nb, Tl, Sl, Dl = cfg["nb"], cfg["T"], cfg["S"], cfg["D"]
    strides, hd = cfg["strides"], cfg["hd"]
    assert strides[1] == 2 * strides[0] and strides[3] == 2 * strides[2]
    HP = 4 * hd  # packed head rows (256)
    s4 = strides[2]

    nc = bacc.Bacc(
        "TRN2",
        target_bir_lowering=False,
        debug=False,
        enable_asserts=False,
        num_devices=N_CORES,
    )

    xT = nc.dram_tensor("xT", [Dl, nb * Tl], MM_DT, kind="ExternalInput").ap()
    eT = nc.dram_tensor("eT", [Dl, nb * Sl], MM_DT, kind="ExternalInput").ap()
    eT4 = nc.dram_tensor(
        "eT4", [Dl, nb * (Sl // s4)], MM_DT, kind="ExternalInput"
    ).ap()
    wqT = nc.dram_tensor("wqT", [Dl, HP], MM_DT, kind="ExternalInput").ap()
    wkT = nc.dram_tensor("wkT", [Dl, HP], MM_DT, kind="ExternalInput").ap()
    wvT = nc.dram_tensor("wvT", [Dl, HP], MM_DT, kind="ExternalInput").ap()
    woT = nc.dram_tensor("woT", [HP, Dl], MM_DT, kind="ExternalInput").ap()
    biases = nc.dram_tensor("biases", [6, 128, 1], F32, kind="ExternalInput").ap()
    out = nc.dram_tensor("partial", [nb * Tl, Dl], F32, kind="ExternalOutput").ap()

    with tile.TileContext(nc) as tc:
        _build_tile(tc, cfg, xT, eT, eT4, wqT, wkT, wvT, woT, biases, out)

    nc.compile()
    return nc


def _build_tile(tc, cfg, xT, eT, eT4, wqT, wkT, wvT, woT, biases, out):
    nc = tc.nc
    nb, Tl, Sl, Dl = cfg["nb"], cfg["T"], cfg["S"], cfg["D"]
    strides, hd = cfg["strides"], cfg["hd"]
    blk, tt = cfg["blk"], cfg["tt"]
    ndc = Dl // 128
    nblk = Sl // blk
    ntt = Tl // tt
    assert ntt == 2, "wide PSUM tiles assume T == 2*tt"
    HP = 4 * hd
    scale = 1.0 / float(np.sqrt(hd))

    from contextlib import ExitStack

    with ExitStack() as ctx:
        wpool = ctx.enter_context(tc.tile_pool(name="weights", bufs=1))
        qtpool = ctx.enter_context(tc.tile_pool(name="qt", bufs=1))
        etpool = ctx.enter_context(tc.tile_pool(name="et", bufs=2))
        ktpool = ctx.enter_context(tc.tile_pool(name="kt", bufs=2))
        vtpool = ctx.enter_context(tc.tile_pool(name="vtT", bufs=2))
        vpool = ctx.enter_context(tc.tile_pool(name="v", bufs=2))
        ppool = ctx.enter_context(tc.tile_pool(name="p", bufs=3))
        npool = ctx.enter_context(tc.tile_pool(name="norm", bufs=2))
        otpool = ctx.enter_context(tc.tile_pool(name="ot", bufs=2))
        obpool = ctx.enter_context(tc.tile_pool(name="outs", bufs=3))
        # PSUM: sc/kv are bank-pair wide ([128, 2*tt] fp32 = 2 banks each),
        # av holds 4 single-bank accumulators -> 8 banks total.
        sc_ps = ctx.enter_context(tc.tile_pool(name="sc_ps", bufs=1, space="PSUM"))
        kv_ps = ctx.enter_context(tc.tile_pool(name="kv_ps", bufs=1, space="PSUM"))
        av_ps = ctx.enter_context(tc.tile_pool(name="av_ps", bufs=1, space="PSUM"))

        # ---- weights into SBUF (one DMA per tensor) ----
        wq_sb = wpool.tile([128, ndc * HP], FR, tag="wq", name="wq_sb")
        wk_sb = wpool.tile([128, ndc * HP], FR, tag="wk", name="wk_sb")
        wv_sb = wpool.tile([128, ndc * HP], FR, tag="wv", name="wv_sb")
        wo_sb = wpool.tile([128, 2 * Dl], FR, tag="wo", name="wo_sb")
        bias_sb = wpool.tile([128, 6], F32, tag="bias", name="bias_sb")
        ones_sb = wpool.tile([128, 1], F32, tag="ones", name="ones_sb")

        def wslice(wsb, dc, p):
            return wsb[:, dc * HP + p * 128 : dc * HP + (p + 1) * 128]

        nc.sync.dma_start(
            out=wq_sb.rearrange("p (c h) -> p c h", c=ndc),
            in_=wqT.rearrange("(c p) h -> p c h", p=128),
        )
        xts = {}
        with tc.tile_pool(name="xt", bufs=1) as xpool:
            for b in range(nb):
                xt = xpool.tile([128, ndc * Tl], FR, tag=f"xt{b}", name="xt")
                nc.sync.dma_start(
                    out=xt.rearrange("p (c t) -> p c t", c=ndc),
                    in_=xT[:, b * Tl : (b + 1) * Tl].rearrange(
                        "(c p) t -> p c t", p=128
                    ),
                )
                xts[b] = xt
            nc.sync.dma_start(
                out=wk_sb.rearrange("p (c h) -> p c h", c=ndc),
                in_=wkT.rearrange("(c p) h -> p c h", p=128),
            )
            nc.sync.dma_start(
                out=wv_sb.rearrange("p (c h) -> p c h", c=ndc),
                in_=wvT.rearrange("(c p) h -> p c h", p=128),
            )
            nc.sync.dma_start(
                out=wo_sb.rearrange("p (g d) -> p g d", g=2),
                in_=woT.rearrange("(g p) d -> p g d", p=128),
            )
            nc.sync.dma_start(
                out=bias_sb, in_=biases.rearrange("g p one -> p (g one)")
            )
            nc.vector.memset(ones_sb, 1.0)

            # ---- phase 1: Q^T = (x @ Wq.T + bq)^T, head pairs on partitions
            qt_sb = {}  # (b, pair) -> [128, T] tile
            for b in range(nb):
                for p in range(2):
                    qt = qtpool.tile([128, Tl], FR, tag=f"qt{b}{p}", name="qt")
                    qt_sb[(b, p)] = qt
                    ps = sc_ps.tile([128, 2 * tt], F32, tag="sc", name="sc_psum")
                    for nt in range(ntt):
                        for dc in range(ndc):
                            _mm(
                                nc,
                                ps[:, nt * tt : (nt + 1) * tt],
                                wslice(wq_sb, dc, p),
                                xts[b][:, dc * Tl + nt * tt : dc * Tl + (nt + 1) * tt],
                                start=(dc == 0),
                                stop=(dc == ndc - 1),
                            )
                    nc.scalar.activation(
                        qt, ps, AF.Identity, bias=bias_sb[:, p : p + 1]
                    )

        # ---- phase 2: attention per (batch, pair), out proj per batch ----
        ot_sb = {}
        for b in range(nb):
            for p in range(2):
                sA = strides[2 * p]
                src = eT if p == 0 else eT4
                Scols = Sl // sA  # union columns per batch for this pair
                ublk = blk // sA  # union columns per block
                nA = ublk // 128  # A-key chunks per block
                nB = nA // 2  # B-key chunks per block
                avp = {}
                for h in range(2):
                    for nt in range(ntt):
                        avp[(h, nt)] = av_ps.tile(
                            [hd + 1, tt], F32, tag=f"av{h}{nt}", name="av_psum"
                        )

                pending = []  # AV emissions delayed by one chunk-step

                def flush_pending():
                    for vt, pt, h, first, last in pending:
                        for nt in range(ntt):
                            _mm(
                                nc,
                                avp[(h, nt)],
                                vt,
                                pt[:, nt * tt : (nt + 1) * tt],
                                start=first,
                                stop=last,
                            )
                    pending.clear()

                for ib in range(nblk):
                    et = etpool.tile([128, ndc * ublk], FR, tag="et", name="et_t")
                    c0_ = b * Scols + ib * ublk
                    nc.sync.dma_start(
                        out=et.rearrange("p (c u) -> p c u", c=ndc),
                        in_=src[:, c0_ : c0_ + ublk].rearrange(
                            "(c p) u -> p c u", p=128
                        ),
                    )

                    # K^T proj (packed pair; B compacted to even union cols)
                    kt_A = ktpool.tile([64, ublk], FR, tag="ktA", name="kt_A")
                    kt_B = ktpool.tile([128, ublk // 2], FR, tag="ktB", name="kt_B")
                    vtT_A = vtpool.tile([64, ublk], FR, tag="vtA", name="vtT_A")
                    vtT_B = vtpool.tile([128, ublk // 2], FR, tag="vtB", name="vtT_B")
                    kps = sc_ps.tile([128, 2 * tt], F32, tag="sc", name="sc_psum")
                    for c0 in range(0, ublk, tt):
                        cw = min(tt, ublk - c0)
                        for dc in range(ndc):
                            _mm(
                                nc,
                                kps[:, c0 : c0 + cw],
                                wslice(wk_sb, dc, p),
                                et[:, dc * ublk + c0 : dc * ublk + c0 + cw],
                                start=(dc == 0),
                                stop=(dc == ndc - 1),
                            )
                    nc.vector.tensor_scalar_add(
                        kt_A, kps[0:64, 0:ublk], bias_sb[0:64, 2 + p : 3 + p]
                    )
                    nc.vector.tensor_scalar_add(
                        kt_B[64:128, :],
                        kps[64:128, 0:ublk:2],
                        bias_sb[64:128, 2 + p : 3 + p],
                    )
                    # V^T proj (same streaming, wv stationary)
                    vps = kv_ps.tile([128, 2 * tt], F32, tag="kv", name="kv_psum")
                    for c0 in range(0, ublk, tt):
                        cw = min(tt, ublk - c0)
                        for dc in range(ndc):
                            _mm(
                                nc,
                                vps[:, c0 : c0 + cw],
                                wslice(wv_sb, dc, p),
                                et[:, dc * ublk + c0 : dc * ublk + c0 + cw],
                                start=(dc == 0),
                                stop=(dc == ndc - 1),
                            )
                    nc.vector.tensor_scalar_add(
                        vtT_A, vps[0:64, 0:ublk], bias_sb[0:64, 4 + p : 5 + p]
                    )
                    nc.vector.tensor_scalar_add(
                        vtT_B[64:128, :],
                        vps[64:128, 0:ublk:2],
                        bias_sb[64:128, 4 + p : 5 + p],
                    )

                    # V -> [keys, hd] via per-chunk DMA XBAR transposes.
                    # Chunk pitch 80 elems (160B) keeps every transpose
                    # destination 32B-aligned (xbar address encoding).
                    VP = hd + 16
                    vtA = vpool.tile([128, nA * VP], FR, tag="vA", name="vtA")
                    vtA3 = vtA.rearrange("p (c f) -> p c f", c=nA)
                    nc.vector.memset(vtA3[:, :, hd : hd + 1], 1.0)
                    for ck in range(nA):
                        nc.sync.dma_start(
                            out=vtA3[:, ck, 0:hd],
                            in_=vtT_A[:, ck * 128 : (ck + 1) * 128],
                            transpose=True,
                        )
                    vtB = vpool.tile([128, nB * VP], FR, tag="vB", name="vtB")
                    vtB3 = vtB.rearrange("p (c f) -> p c f", c=nB)
                    nc.vector.memset(vtB3[:, :, hd : hd + 1], 1.0)
                    for ck in range(nB):
                        nc.sync.dma_start(
                            out=vtB3[:, ck, 0:hd],
                            in_=vtT_B[64:128, ck * 128 : (ck + 1) * 128],
                            transpose=True,
                        )

                    # scores + exp + AV, software-pipelined by one chunk
                    for ck in range(nA):
                        do_B = ck % 2 == 1
                        ckb = ck // 2
                        ptA = ppool.tile([128, Tl], FR, tag="pA", name="ptA")
                        sa = sc_ps.tile([128, 2 * tt], F32, tag="sc", name="sc_psum")
                        if do_B:
                            ptB = ppool.tile([128, Tl], FR, tag="pB", name="ptB")
                            sb_ = kv_ps.tile(
                                [128, 2 * tt], F32, tag="kv", name="kv_psum"
                            )
                        for nt in range(ntt):
                            _mm(
                                nc,
                                sa[:, nt * tt : (nt + 1) * tt],
                                kt_A[:, ck * 128 : (ck + 1) * 128],
                                qt_sb[(b, p)][0:64, nt * tt : (nt + 1) * tt],
                                start=True,
                                stop=True,
                            )
                            if do_B:
                                _mm(
                                    nc,
                                    sb_[:, nt * tt : (nt + 1) * tt],
                                    kt_B[64:128, ckb * 128 : (ckb + 1) * 128],
                                    qt_sb[(b, p)][64:128, nt * tt : (nt + 1) * tt],
                                    start=True,
                                    stop=True,
                                )
                        nc.scalar.activation(ptA, sa, AF.Exp, scale=scale)
                        if do_B:
                            nc.scalar.activation(ptB, sb_, AF.Exp, scale=scale)
                        flush_pending()
                        pending.append(
                            (
                                vtA[:, ck * VP : ck * VP + hd + 1],
                                ptA,
                                0,
                                ib == 0 and ck == 0,
                                ib == nblk - 1 and ck == nA - 1,
                            )
                        )
                        if do_B:
                            pending.append(
                                (
                                    vtB[:, ckb * VP : ckb * VP + hd + 1],
                                    ptB,
                                    1,
                                    ib == 0 and ckb == 0,
                                    ib == nblk - 1 and ckb == nB - 1,
                                )
                            )
                flush_pending()

                # ---- normalize: ot rows 0-63 = head A, 64-127 = head B ----
                # Stack the 4 denominators on 4 partitions, one batched
                # reciprocal, then broadcast + multiply per (h, nt).
                ot = otpool.tile([128, Tl], FR, tag=f"ot{p}", name="ot")
                ot_sb[(b, p)] = ot
                for h in range(2):
                    for nt in range(ntt):
                        r0 = npool.tile([1, tt], F32, tag="r0", name="r0")
                        nc.vector.reciprocal(r0, avp[(h, nt)][hd : hd + 1, :])
                        rb = npool.tile([hd, tt], F32, tag="rb", name="rbcast")
                        nc.gpsimd.partition_broadcast(rb, r0)
                        nc.vector.tensor_mul(
                            ot[h * hd : (h + 1) * hd, nt * tt : (nt + 1) * tt],
                            avp[(h, nt)][0:hd, :],
                            rb,
                        )

            # ---- phase 3: output projection for batch b ----
            assert Dl <= 2 * tt
            for tc_i in range(Tl // 128):
                ops = sc_ps.tile([128, 2 * tt], F32, tag="sc", name="sc_psum")
                for j in range(0, Dl, tt):
                    dw = min(tt, Dl - j)
                    for p in range(2):
                        _mm(
                            nc,
                            ops[:, j : j + dw],
                            ot_sb[(b, p)][:, tc_i * 128 : (tc_i + 1) * 128],
                            wo_sb[:, p * Dl + j : p * Dl + j + dw],
                            start=(p == 0),
                            stop=(p == 1),
                        )
                ob = obpool.tile([128, Dl], F32, tag="ob", name="ob")
                nc.vector.tensor_copy(ob, ops[:, 0:Dl])
                nc.sync.dma_start(
                    out=out[b * Tl + tc_i * 128 : b * Tl + (tc_i + 1) * 128, :],
                    in_=ob,
                )


# ---------------------------------------------------------------------------
# Host-side sharding / gathering
# ---------------------------------------------------------------------------


def _core_map():
    """core -> (batches, heads)"""
    m = {}
    for c in range(N_CORES):
        g = c % 4
        bs = [0, 1] if c < 4 else [2, 3]
        hs = [4 * g + i for i in range(4)]
        m[c] = (bs, hs)
    return m


def shard_inputs(inputs, cfg):
    x = np.asarray(inputs["decoder_input"], np.float32)
    e = np.asarray(inputs["encoder_output"], np.float32)
    Wq = np.asarray(inputs["Wq"], np.float32)
    Wk = np.asarray(inputs["Wk"], np.float32)
    Wv = np.asarray(inputs["Wv"], np.float32)
    Wo = np.asarray(inputs["Wo"], np.float32)
    bq = np.asarray(inputs["bq"], np.float32)
    bk = np.asarray(inputs["bk"], np.float32)
    bv = np.asarray(inputs["bv"], np.float32)
    hd = cfg["hd"]
    s4 = cfg["strides"][2]
    in_maps = []
    for c, (bs, hs) in _core_map().items():
        rows = np.concatenate([np.arange(h * hd, (h + 1) * hd) for h in hs])
        xTc = np.ascontiguousarray(
            x[bs].reshape(len(bs) * cfg["T"], cfg["D"]).T.astype(BF16)
        )
        eTc = np.ascontiguousarray(
            e[bs].reshape(len(bs) * cfg["S"], cfg["D"]).T.astype(BF16)
        )
        e4 = e[bs][:, ::s4, :]  # [nb, S//s4, D]
        eT4c = np.ascontiguousarray(
            e4.reshape(len(bs) * (cfg["S"] // s4), cfg["D"]).T.astype(BF16)
        )
        bias = np.stack([bq[rows], bk[rows], bv[rows]]).reshape(6, 128, 1)
        in_maps.append(
            {
                "xT": xTc,
                "eT": eTc,
                "eT4": eT4c,
                "wqT": np.ascontiguousarray(Wq[rows].T.astype(BF16)),
                "wkT": np.ascontiguousarray(Wk[rows].T.astype(BF16)),
                "wvT": np.ascontiguousarray(Wv[rows].T.astype(BF16)),
                "woT": np.ascontiguousarray(Wo[:, rows].T.astype(BF16)),
                "biases": np.ascontiguousarray(bias),
            }
        )
    return in_maps


def gather_output(results, bo, cfg):
    Tl, Dl = cfg["T"], cfg["D"]
    out = np.zeros((B, Tl, Dl), np.float32)
    for c, (bs, _hs) in _core_map().items():
        p = results[c]["partial"].reshape(len(bs), Tl, Dl)
        for i, b in enumerate(bs):
            out[b] += p[i]
    return out + np.asarray(bo, np.float32)[None, None, :]


_COMPILED = None


def _get_compiled():
    global _COMPILED
    if _COMPILED is None:
        _COMPILED = build_program(FULL_CFG)
    return _COMPILED


def run_on_cores(inputs, trace=False, **kw):
    nc = _get_compiled()
    in_maps = shard_inputs(inputs, FULL_CFG)
    res = bass_utils.run_bass_kernel_spmd(
        nc, in_maps, core_ids=list(range(N_CORES)), trace=trace, **kw
    )
    return res


def kernel(**inputs) -> np.ndarray:
    res = run_on_cores(inputs, trace=False)
    return gather_output(res.results, inputs["bo"], FULL_CFG)


# revision 11
# speedup vs baseline: 1.2525x; 1.0131x over previous
# HEPOS cross-attention (strided per-head K/V) on 8 Trainium2 NeuronCores.
#
# Reference computation (per head h, stride s = STRIDE_LIST[h]):
#   Q = x @ Wq.T + bq ; K = e @ Wk.T + bk ; V = e @ Wv.T + bv
#   out_h = softmax(Q_h @ K_h[::s].T / 8) @ V_h[::s]
#   out   = concat_h(out_h) @ Wo.T + bo
#
# Sharding: 64 (batch, head) units over 8 cores. Core c owns head group
# g = c % 4 (heads 4g..4g+3, strides [1,2,4,8]) and batch pair [0,1]
# (c < 4) or [2,3] (c >= 4). Each core computes its heads' contribution
# to out; the host sums the four partials per batch and adds bo.
#
# On-device design (v3):
#  * Heads are processed as two stride PAIRS (sA, 2*sA): (1,2) and (4,8).
#    Head A of a pair lives on SBUF partitions 0-63, head B on 64-127.
#  * Q/K/V projections use the weight matrix as the matmul stationary with
#    both heads packed into the 128 stationary columns (full PE width).
#    K^T/V^T stream the "union" encoder columns (stride sA); head B rows
#    are valid at even union columns and are compacted on evacuation.
#    The stride-4 union for pair (4,8) is pre-packed by the host (eT4).
#  * V^T ([hd, keys]) is flipped to AV orientation ([keys, hd]) with ONE
#    DMA XBAR transpose per (block, head) (3D output access pattern) -
#    zero PE cost, one sync-engine dispatch each.
#  * Scores are computed transposed ([keys, T]); head B's score matmuls
#    use partitions 64-127 (PE row-tile T8) and overlap head A's (T0).
#  * AV accumulates into PSUM tiles resident across all encoder blocks of
#    a (batch, pair); the softmax denominator falls out of a ones-column
#    appended to the V stationary.
#  * Scores of chunk k+1 are issued before AV of chunk k so the PE never
#    waits on the scalar engine's exp.
#  * All DRAM->SBUF loads are single merged DMAs ([128, ndc, *] access
#    patterns); PSUM score tiles are bank-pair wide ([128, 2*tt]) so exp
#    and evacuations run as one instruction per tile.

import os
import sys

import ml_dtypes
import numpy as np

BF16 = ml_dtypes.bfloat16

for _p in ("/opt/trn_rl_repo", "/root/.axon_site/_ro/trn_rl_repo"):
    if os.path.isdir(_p) and _p not in sys.path:
        sys.path.insert(0, _p)

import concourse.bass as bass  # noqa: E402
import concourse.tile as tile  # noqa: E402
from concourse import bacc, mybir  # noqa: E402
from concourse import bass_utils  # noqa: E402

F32 = mybir.dt.float32
MM_DT = mybir.dt.bfloat16  # matmul operand dtype: full PE rate, half DMA
AF = mybir.ActivationFunctionType

D_MODEL = 1024
NUM_HEADS = 16
HEAD_DIM = 64
STRIDE_LIST = [1, 2, 4, 8] * 4
B, T, S = 4, 1024, 4096
N_CORES = 8

FULL_CFG = dict(
    nb=2,  # batches per core
    T=T,
    S=S,
    D=D_MODEL,
    strides=(1, 2, 4, 8),  # per-core head strides; pairs (s0,s1),(s2,s3)
    hd=HEAD_DIM,
    blk=1024,  # encoder S-block (stride-1 columns) per iteration
    tt=512,  # T tile (PSUM free-dim limit for fp32)
)

FR = MM_DT


def _mm(nc, out, lhsT, rhs, start, stop):
    nc.tensor.matmul(out, lhsT, rhs, start=start, stop=stop)


def build_program(cfg):
    """Build the per-core Bass/Tile program (same program on all cores)."""
    nb, Tl, Sl, Dl = cfg["nb"], cfg["T"], cfg["S"], cfg["D"]
    strides, hd = cfg["strides"], cfg["hd"]
    assert strides[1] == 2 * strides[0] and strides[3] == 2 * strides[2]
    HP = 4 * hd  # packed head rows (256)
    s4 = strides[2]

    nc = bacc.Bacc(
        "TRN2",
        target_bir_lowering=False,
        debug=False,
        enable_asserts=False,
        num_devices=N_CORES,
    )

    xT = nc.dram_tensor("xT", [Dl, nb * Tl], MM_DT, kind="ExternalInput").ap()
    eT = nc.dram_tensor("eT", [Dl, nb * Sl], MM_DT, kind="ExternalInput").ap()
    eT4 = nc.dram_tensor(
        "eT4", [Dl, nb * (Sl // s4)], MM_DT, kind="ExternalInput"
    ).ap()
    wqT = nc.dram_tensor("wqT", [Dl, HP], MM_DT, kind="ExternalInput").ap()
    wkT = nc.dram_tensor("wkT", [Dl, HP], MM_DT, kind="ExternalInput").ap()
    wvT = nc.dram_tensor("wvT", [Dl, HP], MM_DT, kind="ExternalInput").ap()
    woT = nc.dram_tensor("woT", [HP, Dl], MM_DT, kind="ExternalInput").ap()
    biases = nc.dram_tensor("biases", [6, 128, 1], F32, kind="ExternalInput").ap()
    out = nc.dram_tensor("partial", [nb * Tl, Dl], F32, kind="ExternalOutput").ap()

    with tile.TileContext(nc) as tc:
        _build_tile(tc, cfg, xT, eT, eT4, wqT, wkT, wvT, woT, biases, out)

    nc.compile()
    return nc


def _build_tile(tc, cfg, xT, eT, eT4, wqT, wkT, wvT, woT, biases, out):
    nc = tc.nc
    nb, Tl, Sl, Dl = cfg["nb"], cfg["T"], cfg["S"], cfg["D"]
    strides, hd = cfg["strides"], cfg["hd"]
    blk, tt = cfg["blk"], cfg["tt"]
    ndc = Dl // 128
    nblk = Sl // blk
    ntt = Tl // tt
    assert ntt == 2, "wide PSUM tiles assume T == 2*tt"
    HP = 4 * hd
    scale = 1.0 / float(np.sqrt(hd))

    from contextlib import ExitStack

    with ExitStack() as ctx:
        wpool = ctx.enter_context(tc.tile_pool(name="weights", bufs=1))
        qtpool = ctx.enter_context(tc.tile_pool(name="qt", bufs=1))
        etpool = ctx.enter_context(tc.tile_pool(name="et", bufs=2))
        ktpool = ctx.enter_context(tc.tile_pool(name="kt", bufs=2))
        vtpool = ctx.enter_context(tc.tile_pool(name="vtT", bufs=2))
        vpool = ctx.enter_context(tc.tile_pool(name="v", bufs=2))
        ppool = ctx.enter_context(tc.tile_pool(name="p", bufs=3))
        npool = ctx.enter_context(tc.tile_pool(name="norm", bufs=2))
        otpool = ctx.enter_context(tc.tile_pool(name="ot", bufs=2))
        obpool = ctx.enter_context(tc.tile_pool(name="outs", bufs=3))
        # PSUM: sc/kv are bank-pair wide ([128, 2*tt] fp32 = 2 banks each),
        # av holds 4 single-bank accumulators -> 8 banks total.
        sc_ps = ctx.enter_context(tc.tile_pool(name="sc_ps", bufs=1, space="PSUM"))
        kv_ps = ctx.enter_context(tc.tile_pool(name="kv_ps", bufs=1, space="PSUM"))
        av_ps = ctx.enter_context(tc.tile_pool(name="av_ps", bufs=1, space="PSUM"))

        # ---- weights into SBUF (one DMA per tensor) ----
        wq_sb = wpool.tile([128, ndc * HP], FR, tag="wq", name="wq_sb")
        wk_sb = wpool.tile([128, ndc * HP], FR, tag="wk", name="wk_sb")
        wv_sb = wpool.tile([128, ndc * HP], FR, tag="wv", name="wv_sb")
        wo_sb = wpool.tile([128, 2 * Dl], FR, tag="wo", name="wo_sb")
        bias_sb = wpool.tile([128, 6], F32, tag="bias", name="bias_sb")
        ones_sb = wpool.tile([128, 1], F32, tag="ones", name="ones_sb")

        def wslice(wsb, dc, p):
            return wsb[:, dc * HP + p * 128 : dc * HP + (p + 1) * 128]

        # encoder block list + DMA helper (defined early so the first
        # block's load can be interleaved with the weight loads)
        blocks = [(b, p, ib) for b in range(nb) for p in range(2) for ib in range(nblk)]

        def block_params(p):
            sA = strides[2 * p]
            return dict(
                src=eT if p == 0 else eT4,
                Scols=Sl // sA,
                ublk=blk // sA,
            )

        def emit_et_dma(b, p, ib):
            bp = block_params(p)
            ublk = bp["ublk"]
            et = etpool.tile([128, ndc * ublk], FR, tag="et", name="et_t")
            c0_ = b * bp["Scols"] + ib * ublk
            nc.sync.dma_start(
                out=et.rearrange("p (c u) -> p c u", c=ndc),
                in_=bp["src"][:, c0_ : c0_ + ublk].rearrange(
                    "(c p) u -> p c u", p=128
                ),
            )
            return et

        nc.sync.dma_start(
            out=wq_sb.rearrange("p (c h) -> p c h", c=ndc),
            in_=wqT.rearrange("(c p) h -> p c h", p=128),
        )
        xts = {}
        with tc.tile_pool(name="xt", bufs=1) as xpool:
            for b in range(nb):
                xt = xpool.tile([128, ndc * Tl], FR, tag=f"xt{b}", name="xt")
                xts[b] = xt
            nc.sync.dma_start(
                out=xts[0].rearrange("p (c t) -> p c t", c=ndc),
                in_=xT[:, 0:Tl].rearrange("(c p) t -> p c t", p=128),
            )
            nc.sync.dma_start(
                out=wk_sb.rearrange("p (c h) -> p c h", c=ndc),
                in_=wkT.rearrange("(c p) h -> p c h", p=128),
            )
            et_next = emit_et_dma(*blocks[0])
            nc.sync.dma_start(
                out=wv_sb.rearrange("p (c h) -> p c h", c=ndc),
                in_=wvT.rearrange("(c p) h -> p c h", p=128),
            )
            for b in range(1, nb):
                nc.sync.dma_start(
                    out=xts[b].rearrange("p (c t) -> p c t", c=ndc),
                    in_=xT[:, b * Tl : (b + 1) * Tl].rearrange(
                        "(c p) t -> p c t", p=128
                    ),
                )
            nc.sync.dma_start(
                out=wo_sb.rearrange("p (g d) -> p g d", g=2),
                in_=woT.rearrange("(g p) d -> p g d", p=128),
            )
            nc.sync.dma_start(
                out=bias_sb, in_=biases.rearrange("g p one -> p (g one)")
            )
            nc.vector.memset(ones_sb, 1.0)

            # ---- phase 1: Q^T = (x @ Wq.T + bq)^T, head pairs on partitions
            qt_sb = {}  # (b, pair) -> [128, T] tile
            for b in range(nb):
                for p in range(2):
                    qt = qtpool.tile([128, Tl], FR, tag=f"qt{b}{p}", name="qt")
                    qt_sb[(b, p)] = qt
                    ps = sc_ps.tile([128, 2 * tt], F32, tag="sc", name="sc_psum")
                    for nt in range(ntt):
                        for dc in range(ndc):
                            _mm(
                                nc,
                                ps[:, nt * tt : (nt + 1) * tt],
                                wslice(wq_sb, dc, p),
                                xts[b][:, dc * Tl + nt * tt : dc * Tl + (nt + 1) * tt],
                                start=(dc == 0),
                                stop=(dc == ndc - 1),
                            )
                    nc.scalar.activation(
                        qt, ps, AF.Identity, bias=bias_sb[:, p : p + 1]
                    )

        # ---- phase 2: attention per (batch, pair), out proj per batch ----
        # Blocks are processed as a flat list with the next block's encoder
        # DMA dispatched BEFORE this block's V transposes (keeps the sync
        # queue from head-of-line blocking the PE), and each batch's output
        # projection deferred until after the next batch's first block (so
        # it never waits on the normalize chain).
        assert Dl <= 2 * tt
        ot_sb = {}
        avp_live = {}  # (b, p) -> avp dict
        pending = []  # AV emissions delayed by one chunk-step

        def flush_pending():
            for avp, vt, pt, h, first, last in pending:
                for nt in range(ntt):
                    _mm(
                        nc,
                        avp[(h, nt)],
                        vt,
                        pt[:, nt * tt : (nt + 1) * tt],
                        start=first,
                        stop=last,
                    )
            pending.clear()

        def emit_normalize(b, p):
            """avp PSUM -> normalized ot (rows 0-63 head A, 64-127 head B)."""
            avp = avp_live.pop((b, p))
            ot = otpool.tile([128, Tl], FR, tag=f"ot{p}", name="ot")
            ot_sb[(b, p)] = ot
            for nt in range(ntt):
                for h in range(2):
                    r0 = npool.tile([1, tt], F32, tag="r0", name="r0")
                    nc.vector.reciprocal(r0, avp[(h, nt)][hd : hd + 1, :])
                    rb = npool.tile([hd, tt], F32, tag="rb", name="rbcast")
                    nc.gpsimd.partition_broadcast(rb, r0)
                    nc.vector.tensor_mul(
                        ot[h * hd : (h + 1) * hd, nt * tt : (nt + 1) * tt],
                        avp[(h, nt)][0:hd, :],
                        rb,
                    )

        def emit_out_proj(b):
            for tc_i in range(Tl // 128):
                ops = sc_ps.tile([128, 2 * tt], F32, tag="sc", name="sc_psum")
                for j in range(0, Dl, tt):
                    dw = min(tt, Dl - j)
                    for p in range(2):
                        _mm(
                            nc,
                            ops[:, j : j + dw],
                            ot_sb[(b, p)][:, tc_i * 128 : (tc_i + 1) * 128],
                            wo_sb[:, p * Dl + j : p * Dl + j + dw],
                            start=(p == 0),
                            stop=(p == 1),
                        )
                ob = obpool.tile([128, Dl], F32, tag="ob", name="ob")
                nc.vector.tensor_copy(ob, ops[:, 0:Dl])
                nc.sync.dma_start(
                    out=out[b * Tl + tc_i * 128 : b * Tl + (tc_i + 1) * 128, :],
                    in_=ob,
                )

        for bi, (b, p, ib) in enumerate(blocks):
            bp = block_params(p)
            ublk = bp["ublk"]
            nA = ublk // 128
            nB = nA // 2
            et = et_next
            if (b, p) not in avp_live:
                avp_live[(b, p)] = {
                    (h, nt): av_ps.tile(
                        [hd + 1, tt], F32, tag=f"av{h}{nt}", name="av_psum"
                    )
                    for h in range(2)
                    for nt in range(ntt)
                }
            avp = avp_live[(b, p)]

            # K^T proj (packed pair; B compacted to even union cols)
            kt_A = ktpool.tile([64, ublk], FR, tag="ktA", name="kt_A")
            kt_B = ktpool.tile([128, ublk // 2], FR, tag="ktB", name="kt_B")
            vtT_A = vtpool.tile([64, ublk], FR, tag="vtA", name="vtT_A")
            vtT_B = vtpool.tile([128, ublk // 2], FR, tag="vtB", name="vtT_B")
            kps = sc_ps.tile([128, 2 * tt], F32, tag="sc", name="sc_psum")
            for c0 in range(0, ublk, tt):
                cw = min(tt, ublk - c0)
                for dc in range(ndc):
                    _mm(
                        nc,
                        kps[:, c0 : c0 + cw],
                        wslice(wk_sb, dc, p),
                        et[:, dc * ublk + c0 : dc * ublk + c0 + cw],
                        start=(dc == 0),
                        stop=(dc == ndc - 1),
                    )
            nc.vector.tensor_scalar_add(
                kt_A, kps[0:64, 0:ublk], bias_sb[0:64, 2 + p : 3 + p]
            )
            nc.vector.tensor_scalar_add(
                kt_B[64:128, :],
                kps[64:128, 0:ublk:2],
                bias_sb[64:128, 2 + p : 3 + p],
            )
            # V^T proj (same streaming, wv stationary)
            vps = kv_ps.tile([128, 2 * tt], F32, tag="kv", name="kv_psum")
            for c0 in range(0, ublk, tt):
                cw = min(tt, ublk - c0)
                for dc in range(ndc):
                    _mm(
                        nc,
                        vps[:, c0 : c0 + cw],
                        wslice(wv_sb, dc, p),
                        et[:, dc * ublk + c0 : dc * ublk + c0 + cw],
                        start=(dc == 0),
                        stop=(dc == ndc - 1),
                    )
            nc.vector.tensor_scalar_add(
                vtT_A, vps[0:64, 0:ublk], bias_sb[0:64, 4 + p : 5 + p]
            )
            nc.vector.tensor_scalar_add(
                vtT_B[64:128, :],
                vps[64:128, 0:ublk:2],
                bias_sb[64:128, 4 + p : 5 + p],
            )

            # prefetch the NEXT block's encoder tile before the transposes
            if bi + 1 < len(blocks):
                et_next = emit_et_dma(*blocks[bi + 1])

            # V -> [keys, hd] via per-chunk DMA XBAR transposes, dispatched
            # in consumption order (A0 A1 B0 A2 A3 B1 ...). Chunk pitch 80
            # elems (160B) keeps destinations 32B-aligned (xbar encoding).
            VP = hd + 16
            vtA = vpool.tile([128, nA * VP], FR, tag="vA", name="vtA")
            vtA3 = vtA.rearrange("p (c f) -> p c f", c=nA)
            nc.vector.memset(vtA3[:, :, hd : hd + 1], 1.0)
            vtB = vpool.tile([128, nB * VP], FR, tag="vB", name="vtB")
            vtB3 = vtB.rearrange("p (c f) -> p c f", c=nB)
            nc.vector.memset(vtB3[:, :, hd : hd + 1], 1.0)
            for ck in range(nA):
                nc.sync.dma_start(
                    out=vtA3[:, ck, 0:hd],
                    in_=vtT_A[:, ck * 128 : (ck + 1) * 128],
                    transpose=True,
                )
                if ck % 2 == 1:
                    ckb = ck // 2
                    nc.sync.dma_start(
                        out=vtB3[:, ckb, 0:hd],
                        in_=vtT_B[64:128, ckb * 128 : (ckb + 1) * 128],
                        transpose=True,
                    )

            # scores + exp + AV, software-pipelined by one chunk
            for ck in range(nA):
                do_B = ck % 2 == 1
                ckb = ck // 2
                ptA = ppool.tile([128, Tl], FR, tag="pA", name="ptA")
                sa = sc_ps.tile([128, 2 * tt], F32, tag="sc", name="sc_psum")
                if do_B:
                    ptB = ppool.tile([128, Tl], FR, tag="pB", name="ptB")
                    sb_ = kv_ps.tile([128, 2 * tt], F32, tag="kv", name="kv_psum")
                for nt in range(ntt):
                    _mm(
                        nc,
                        sa[:, nt * tt : (nt + 1) * tt],
                        kt_A[:, ck * 128 : (ck + 1) * 128],
                        qt_sb[(b, p)][0:64, nt * tt : (nt + 1) * tt],
                        start=True,
                        stop=True,
                    )
                    if do_B:
                        _mm(
                            nc,
                            sb_[:, nt * tt : (nt + 1) * tt],
                            kt_B[64:128, ckb * 128 : (ckb + 1) * 128],
                            qt_sb[(b, p)][64:128, nt * tt : (nt + 1) * tt],
                            start=True,
                            stop=True,
                        )
                nc.scalar.activation(ptA, sa, AF.Exp, scale=scale)
                if do_B:
                    nc.scalar.activation(ptB, sb_, AF.Exp, scale=scale)
                flush_pending()
                pending.append(
                    (
                        avp,
                        vtA[:, ck * VP : ck * VP + hd + 1],
                        ptA,
                        0,
                        ib == 0 and ck == 0,
                        ib == nblk - 1 and ck == nA - 1,
                    )
                )
                if do_B:
                    pending.append(
                        (
                            avp,
                            vtB[:, ckb * VP : ckb * VP + hd + 1],
                            ptB,
                            1,
                            ib == 0 and ckb == 0,
                            ib == nblk - 1 and ckb == nB - 1,
                        )
                    )

            if ib == nblk - 1:
                # pair finished: flush, then normalize (off the PE path)
                flush_pending()
                emit_normalize(b, p)
                if p == 1 and b > 0:
                    emit_out_proj(b - 1)  # deferred from previous batch
        emit_out_proj(nb - 1)

# ---------------------------------------------------------------------------
# Host-side sharding / gathering
# ---------------------------------------------------------------------------


def _core_map():
    """core -> (batches, heads)"""
    m = {}
    for c in range(N_CORES):
        g = c % 4
        bs = [0, 1] if c < 4 else [2, 3]
        hs = [4 * g + i for i in range(4)]
        m[c] = (bs, hs)
    return m


def shard_inputs(inputs, cfg):
    x = np.asarray(inputs["decoder_input"], np.float32)
    e = np.asarray(inputs["encoder_output"], np.float32)
    Wq = np.asarray(inputs["Wq"], np.float32)
    Wk = np.asarray(inputs["Wk"], np.float32)
    Wv = np.asarray(inputs["Wv"], np.float32)
    Wo = np.asarray(inputs["Wo"], np.float32)
    bq = np.asarray(inputs["bq"], np.float32)
    bk = np.asarray(inputs["bk"], np.float32)
    bv = np.asarray(inputs["bv"], np.float32)
    hd = cfg["hd"]
    s4 = cfg["strides"][2]
    in_maps = []
    for c, (bs, hs) in _core_map().items():
        rows = np.concatenate([np.arange(h * hd, (h + 1) * hd) for h in hs])
        xTc = np.ascontiguousarray(
            x[bs].reshape(len(bs) * cfg["T"], cfg["D"]).T.astype(BF16)
        )
        eTc = np.ascontiguousarray(
            e[bs].reshape(len(bs) * cfg["S"], cfg["D"]).T.astype(BF16)
        )
        e4 = e[bs][:, ::s4, :]  # [nb, S//s4, D]
        eT4c = np.ascontiguousarray(
            e4.reshape(len(bs) * (cfg["S"] // s4), cfg["D"]).T.astype(BF16)
        )
        bias = np.stack([bq[rows], bk[rows], bv[rows]]).reshape(6, 128, 1)
        in_maps.append(
            {
                "xT": xTc,
                "eT": eTc,
                "eT4": eT4c,
                "wqT": np.ascontiguousarray(Wq[rows].T.astype(BF16)),
                "wkT": np.ascontiguousarray(Wk[rows].T.astype(BF16)),
                "wvT": np.ascontiguousarray(Wv[rows].T.astype(BF16)),
                "woT": np.ascontiguousarray(Wo[:, rows].T.astype(BF16)),
                "biases": np.ascontiguousarray(bias),
            }
        )
    return in_maps


def gather_output(results, bo, cfg):
    Tl, Dl = cfg["T"], cfg["D"]
    out = np.zeros((B, Tl, Dl), np.float32)
    for c, (bs, _hs) in _core_map().items():
        p = results[c]["partial"].reshape(len(bs), Tl, Dl)
        for i, b in enumerate(bs):
            out[b] += p[i]
    return out + np.asarray(bo, np.float32)[None, None, :]


_COMPILED = None


def _get_compiled():
    global _COMPILED
    if _COMPILED is None:
        _COMPILED = build_program(FULL_CFG)
    return _COMPILED


def run_on_cores(inputs, trace=False, **kw):
    nc = _get_compiled()
    in_maps = shard_inputs(inputs, FULL_CFG)
    res = bass_utils.run_bass_kernel_spmd(
        nc, in_maps, core_ids=list(range(N_CORES)), trace=trace, **kw
    )
    return res


def kernel(**inputs) -> np.ndarray:
    res = run_on_cores(inputs, trace=False)
    return gather_output(res.results, inputs["bo"], FULL_CFG)


# revision 13
# speedup vs baseline: 1.4191x; 1.1330x over previous
# HEPOS cross-attention (strided per-head K/V) on 8 Trainium2 NeuronCores.
#
# Reference computation (per head h, stride s = STRIDE_LIST[h]):
#   Q = x @ Wq.T + bq ; K = e @ Wk.T + bk ; V = e @ Wv.T + bv
#   out_h = softmax(Q_h @ K_h[::s].T / 8) @ V_h[::s]
#   out   = concat_h(out_h) @ Wo.T + bo
#
# Sharding: 64 (batch, head) units over 8 cores. Core c owns head group
# g = c % 4 (heads 4g..4g+3, strides [1,2,4,8]) and batch pair [0,1]
# (c < 4) or [2,3] (c >= 4). Each core computes its heads' contribution
# to out; the host sums the four partials per batch and adds bo.
#
# On-device design (v3):
#  * Heads are processed as two stride PAIRS (sA, 2*sA): (1,2) and (4,8).
#    Head A of a pair lives on SBUF partitions 0-63, head B on 64-127.
#  * Q/K/V projections use the weight matrix as the matmul stationary with
#    both heads packed into the 128 stationary columns (full PE width).
#    K^T/V^T stream the "union" encoder columns (stride sA); head B rows
#    are valid at even union columns and are compacted on evacuation.
#    The stride-4 union for pair (4,8) is pre-packed by the host (eT4).
#  * V^T ([hd, keys]) is flipped to AV orientation ([keys, hd]) with ONE
#    DMA XBAR transpose per (block, head) (3D output access pattern) -
#    zero PE cost, one sync-engine dispatch each.
#  * Scores are computed transposed ([keys, T]); head B's score matmuls
#    use partitions 64-127 (PE row-tile T8) and overlap head A's (T0).
#  * AV accumulates into PSUM tiles resident across all encoder blocks of
#    a (batch, pair); the softmax denominator falls out of a ones-column
#    appended to the V stationary.
#  * Scores of chunk k+1 are issued before AV of chunk k so the PE never
#    waits on the scalar engine's exp.
#  * All DRAM->SBUF loads are single merged DMAs ([128, ndc, *] access
#    patterns); PSUM score tiles are bank-pair wide ([128, 2*tt]) so exp
#    and evacuations run as one instruction per tile.

import os
import sys

import ml_dtypes
import numpy as np

BF16 = ml_dtypes.bfloat16

for _p in ("/opt/trn_rl_repo", "/root/.axon_site/_ro/trn_rl_repo"):
    if os.path.isdir(_p) and _p not in sys.path:
        sys.path.insert(0, _p)

import concourse.bass as bass  # noqa: E402
import concourse.tile as tile  # noqa: E402
from concourse import bacc, mybir  # noqa: E402
from concourse import bass_utils  # noqa: E402

F32 = mybir.dt.float32
MM_DT = mybir.dt.bfloat16  # matmul operand dtype: full PE rate, half DMA
AF = mybir.ActivationFunctionType

D_MODEL = 1024
NUM_HEADS = 16
HEAD_DIM = 64
STRIDE_LIST = [1, 2, 4, 8] * 4
B, T, S = 4, 1024, 4096
N_CORES = 8

FULL_CFG = dict(
    nb=2,  # batches per core
    T=T,
    S=S,
    D=D_MODEL,
    strides=(1, 2, 4, 8),  # per-core head strides; pairs (s0,s1),(s2,s3)
    hd=HEAD_DIM,
    blk=1024,  # encoder S-block (stride-1 columns) per iteration
    tt=512,  # T tile (PSUM free-dim limit for fp32)
)

FR = MM_DT


def _mm(nc, out, lhsT, rhs, start, stop):
    nc.tensor.matmul(out, lhsT, rhs, start=start, stop=stop)


def build_program(cfg):
    """Build the per-core Bass/Tile program (same program on all cores)."""
    nb, Tl, Sl, Dl = cfg["nb"], cfg["T"], cfg["S"], cfg["D"]
    strides, hd = cfg["strides"], cfg["hd"]
    assert strides[1] == 2 * strides[0] and strides[3] == 2 * strides[2]
    HP = 4 * hd  # packed head rows (256)
    s4 = strides[2]

    nc = bacc.Bacc(
        "TRN2",
        target_bir_lowering=False,
        debug=False,
        enable_asserts=False,
        num_devices=N_CORES,
    )

    xT = nc.dram_tensor("xT", [Dl, nb * Tl], MM_DT, kind="ExternalInput").ap()
    eT = nc.dram_tensor("eT", [Dl, nb * Sl], MM_DT, kind="ExternalInput").ap()
    eT4 = nc.dram_tensor(
        "eT4", [Dl, nb * (Sl // s4)], MM_DT, kind="ExternalInput"
    ).ap()
    wqT = nc.dram_tensor("wqT", [Dl, HP], MM_DT, kind="ExternalInput").ap()
    wkT = nc.dram_tensor("wkT", [Dl, HP], MM_DT, kind="ExternalInput").ap()
    wvT = nc.dram_tensor("wvT", [Dl, HP], MM_DT, kind="ExternalInput").ap()
    woT = nc.dram_tensor("woT", [HP, Dl], MM_DT, kind="ExternalInput").ap()
    biases = nc.dram_tensor("biases", [6, 128, 1], F32, kind="ExternalInput").ap()
    out = nc.dram_tensor("partial", [nb * Tl, Dl], F32, kind="ExternalOutput").ap()

    with tile.TileContext(nc) as tc:
        _build_tile(tc, cfg, xT, eT, eT4, wqT, wkT, wvT, woT, biases, out)

    nc.compile()
    return nc


def _build_tile(tc, cfg, xT, eT, eT4, wqT, wkT, wvT, woT, biases, out):
    nc = tc.nc
    nb, Tl, Sl, Dl = cfg["nb"], cfg["T"], cfg["S"], cfg["D"]
    strides, hd = cfg["strides"], cfg["hd"]
    blk, tt = cfg["blk"], cfg["tt"]
    ndc = Dl // 128
    nblk = Sl // blk
    ntt = Tl // tt
    assert ntt == 2, "wide PSUM tiles assume T == 2*tt"
    HP = 4 * hd
    scale = 1.0 / float(np.sqrt(hd))

    from contextlib import ExitStack

    with ExitStack() as ctx:
        wpool = ctx.enter_context(tc.tile_pool(name="weights", bufs=1))
        qtpool = ctx.enter_context(tc.tile_pool(name="qt", bufs=1))
        etpool = ctx.enter_context(tc.tile_pool(name="et", bufs=3))
        ktpool = ctx.enter_context(tc.tile_pool(name="kt", bufs=3))
        vtpool = ctx.enter_context(tc.tile_pool(name="vtT", bufs=2))
        vpool = ctx.enter_context(tc.tile_pool(name="v", bufs=3))
        ppool = ctx.enter_context(tc.tile_pool(name="p", bufs=4))
        npool = ctx.enter_context(tc.tile_pool(name="norm", bufs=2))
        otpool = ctx.enter_context(tc.tile_pool(name="ot", bufs=2))
        obpool = ctx.enter_context(tc.tile_pool(name="outs", bufs=3))
        # PSUM: sc/kv are bank-pair wide ([128, 2*tt] fp32 = 2 banks each),
        # av holds 4 single-bank accumulators -> 8 banks total.
        sc_ps = ctx.enter_context(tc.tile_pool(name="sc_ps", bufs=1, space="PSUM"))
        kv_ps = ctx.enter_context(tc.tile_pool(name="kv_ps", bufs=1, space="PSUM"))
        av_ps = ctx.enter_context(tc.tile_pool(name="av_ps", bufs=1, space="PSUM"))

        # ---- weights into SBUF (one DMA per tensor) ----
        wq_sb = wpool.tile([128, ndc * HP], FR, tag="wq", name="wq_sb")
        wk_sb = wpool.tile([128, ndc * HP], FR, tag="wk", name="wk_sb")
        wv_sb = wpool.tile([128, ndc * HP], FR, tag="wv", name="wv_sb")
        wo_sb = wpool.tile([128, 2 * Dl], FR, tag="wo", name="wo_sb")
        bias_sb = wpool.tile([128, 6], F32, tag="bias", name="bias_sb")
        ones_sb = wpool.tile([128, 1], F32, tag="ones", name="ones_sb")

        def wslice(wsb, dc, p):
            return wsb[:, dc * HP + p * 128 : dc * HP + (p + 1) * 128]

        # encoder block list + DMA helper (defined early so the first
        # block's load can be interleaved with the weight loads)
        blocks = [(b, p, ib) for b in range(nb) for p in range(2) for ib in range(nblk)]

        def block_params(p):
            sA = strides[2 * p]
            return dict(
                src=eT if p == 0 else eT4,
                Scols=Sl // sA,
                ublk=blk // sA,
            )

        def emit_et_dma(b, p, ib):
            bp = block_params(p)
            ublk = bp["ublk"]
            et = etpool.tile([128, ndc * ublk], FR, tag="et", name="et_t")
            c0_ = b * bp["Scols"] + ib * ublk
            nc.sync.dma_start(
                out=et.rearrange("p (c u) -> p c u", c=ndc),
                in_=bp["src"][:, c0_ : c0_ + ublk].rearrange(
                    "(c p) u -> p c u", p=128
                ),
            )
            return et

        nc.sync.dma_start(
            out=wq_sb.rearrange("p (c h) -> p c h", c=ndc),
            in_=wqT.rearrange("(c p) h -> p c h", p=128),
        )
        xts = {}
        with tc.tile_pool(name="xt", bufs=1) as xpool:
            for b in range(nb):
                xt = xpool.tile([128, ndc * Tl], FR, tag=f"xt{b}", name="xt")
                xts[b] = xt
            nc.sync.dma_start(
                out=xts[0].rearrange("p (c t) -> p c t", c=ndc),
                in_=xT[:, 0:Tl].rearrange("(c p) t -> p c t", p=128),
            )
            nc.sync.dma_start(
                out=wk_sb.rearrange("p (c h) -> p c h", c=ndc),
                in_=wkT.rearrange("(c p) h -> p c h", p=128),
            )
            et_next = emit_et_dma(*blocks[0])
            nc.sync.dma_start(
                out=wv_sb.rearrange("p (c h) -> p c h", c=ndc),
                in_=wvT.rearrange("(c p) h -> p c h", p=128),
            )
            for b in range(1, nb):
                nc.sync.dma_start(
                    out=xts[b].rearrange("p (c t) -> p c t", c=ndc),
                    in_=xT[:, b * Tl : (b + 1) * Tl].rearrange(
                        "(c p) t -> p c t", p=128
                    ),
                )
            nc.sync.dma_start(
                out=wo_sb.rearrange("p (g d) -> p g d", g=2),
                in_=woT.rearrange("(g p) d -> p g d", p=128),
            )
            nc.sync.dma_start(
                out=bias_sb, in_=biases.rearrange("g p one -> p (g one)")
            )
            nc.vector.memset(ones_sb, 1.0)

            # ---- phase 1: Q^T = (x @ Wq.T + bq)^T, head pairs on partitions
            qt_sb = {}  # (b, pair) -> [128, T] tile
            for b in range(nb):
                for p in range(2):
                    qt = qtpool.tile([128, Tl], FR, tag=f"qt{b}{p}", name="qt")
                    qt_sb[(b, p)] = qt
                    ps = sc_ps.tile([128, 2 * tt], F32, tag="sc", name="sc_psum")
                    for nt in range(ntt):
                        for dc in range(ndc):
                            _mm(
                                nc,
                                ps[:, nt * tt : (nt + 1) * tt],
                                wslice(wq_sb, dc, p),
                                xts[b][:, dc * Tl + nt * tt : dc * Tl + (nt + 1) * tt],
                                start=(dc == 0),
                                stop=(dc == ndc - 1),
                            )
                    nc.scalar.activation(
                        qt, ps, AF.Identity, bias=bias_sb[:, p : p + 1]
                    )

        # ---- phase 2: attention per (batch, pair), out proj per batch ----
        # The per-block work is split into phase A (K^T/V^T projection,
        # evacuation, V transposes, next-block encoder DMA) and phase B
        # (scores/exp/AV chunk loop), software-pipelined one block deep:
        #   pA(0) pA(1) pB(0) pA(2) pB(1) ... pA(n-1) pB(n-3) pB(n-2) pB(n-1)
        # so V transposes are dispatched a full block before their AV
        # consumes them and the normalize chain never blocks evacuations.
        # AV emission inside phase B additionally lags scores by two chunk
        # steps so the PE never waits on the scalar engine's exp.
        assert Dl <= 2 * tt
        ot_sb = {}
        avp_live = {}
        blk_state = {}
        pending = []  # (age, avp, vt, pt, h, first, last)

        def flush_pending(min_age=2):
            keep = []
            for age, avp, vt, pt, h, first, last in pending:
                if age >= min_age:
                    for nt in range(ntt):
                        _mm(
                            nc,
                            avp[(h, nt)],
                            vt,
                            pt[:, nt * tt : (nt + 1) * tt],
                            start=first,
                            stop=last,
                        )
                else:
                    keep.append((age + 1, avp, vt, pt, h, first, last))
            pending[:] = keep

        def emit_normalize(b, p):
            """avp PSUM -> normalized ot (rows 0-63 head A, 64-127 head B)."""
            avp = avp_live.pop((b, p))
            ot = otpool.tile([128, Tl], FR, tag=f"ot{p}", name="ot")
            ot_sb[(b, p)] = ot
            for nt in range(ntt):
                for h in range(2):
                    r0 = npool.tile([1, tt], F32, tag="r0", name="r0")
                    nc.vector.reciprocal(r0, avp[(h, nt)][hd : hd + 1, :])
                    rb = npool.tile([hd, tt], F32, tag="rb", name="rbcast")
                    nc.gpsimd.partition_broadcast(rb, r0)
                    nc.vector.tensor_mul(
                        ot[h * hd : (h + 1) * hd, nt * tt : (nt + 1) * tt],
                        avp[(h, nt)][0:hd, :],
                        rb,
                    )

        def emit_out_proj(b):
            for tc_i in range(Tl // 128):
                ops = sc_ps.tile([128, 2 * tt], F32, tag="sc", name="sc_psum")
                for j in range(0, Dl, tt):
                    dw = min(tt, Dl - j)
                    for p in range(2):
                        _mm(
                            nc,
                            ops[:, j : j + dw],
                            ot_sb[(b, p)][:, tc_i * 128 : (tc_i + 1) * 128],
                            wo_sb[:, p * Dl + j : p * Dl + j + dw],
                            start=(p == 0),
                            stop=(p == 1),
                        )
                ob = obpool.tile([128, Dl], F32, tag="ob", name="ob")
                nc.vector.tensor_copy(ob, ops[:, 0:Dl])
                nc.sync.dma_start(
                    out=out[b * Tl + tc_i * 128 : b * Tl + (tc_i + 1) * 128, :],
                    in_=ob,
                )

        ets = {0: et_next}

        def phase_a(bi):
            b, p, ib = blocks[bi]
            ublk = block_params(p)["ublk"]
            nA = ublk // 128
            nB = nA // 2
            et = ets.pop(bi)
            # K^T proj (packed pair; B compacted to even union cols)
            kt_A = ktpool.tile([64, ublk], FR, tag="ktA", name="kt_A")
            kt_B = ktpool.tile([128, ublk // 2], FR, tag="ktB", name="kt_B")
            vtT_A = vtpool.tile([64, ublk], FR, tag="vtA", name="vtT_A")
            vtT_B = vtpool.tile([128, ublk // 2], FR, tag="vtB", name="vtT_B")
            kps = sc_ps.tile([128, 2 * tt], F32, tag="sc", name="sc_psum")
            for c0 in range(0, ublk, tt):
                cw = min(tt, ublk - c0)
                for dc in range(ndc):
                    _mm(
                        nc,
                        kps[:, c0 : c0 + cw],
                        wslice(wk_sb, dc, p),
                        et[:, dc * ublk + c0 : dc * ublk + c0 + cw],
                        start=(dc == 0),
                        stop=(dc == ndc - 1),
                    )
            nc.vector.tensor_scalar_add(
                kt_A, kps[0:64, 0:ublk], bias_sb[0:64, 2 + p : 3 + p]
            )
            nc.vector.tensor_scalar_add(
                kt_B[64:128, :],
                kps[64:128, 0:ublk:2],
                bias_sb[64:128, 2 + p : 3 + p],
            )
            # V^T proj (same streaming, wv stationary)
            vps = kv_ps.tile([128, 2 * tt], F32, tag="kv", name="kv_psum")
            for c0 in range(0, ublk, tt):
                cw = min(tt, ublk - c0)
                for dc in range(ndc):
                    _mm(
                        nc,
                        vps[:, c0 : c0 + cw],
                        wslice(wv_sb, dc, p),
                        et[:, dc * ublk + c0 : dc * ublk + c0 + cw],
                        start=(dc == 0),
                        stop=(dc == ndc - 1),
                    )
            nc.vector.tensor_scalar_add(
                vtT_A, vps[0:64, 0:ublk], bias_sb[0:64, 4 + p : 5 + p]
            )
            nc.vector.tensor_scalar_add(
                vtT_B[64:128, :],
                vps[64:128, 0:ublk:2],
                bias_sb[64:128, 4 + p : 5 + p],
            )

            # prefetch the NEXT block's encoder tile before the transposes
            if bi + 1 < len(blocks):
                ets[bi + 1] = emit_et_dma(*blocks[bi + 1])

            # V -> [keys, hd] via per-chunk DMA XBAR transposes, dispatched
            # in consumption order (A0 A1 B0 A2 A3 B1 ...). Chunk pitch 80
            # elems (160B) keeps destinations 32B-aligned (xbar encoding).
            VP = hd + 16
            vtA = vpool.tile([128, nA * VP], FR, tag="vA", name="vtA")
            vtA3 = vtA.rearrange("p (c f) -> p c f", c=nA)
            nc.vector.memset(vtA3[:, :, hd : hd + 1], 1.0)
            vtB = vpool.tile([128, nB * VP], FR, tag="vB", name="vtB")
            vtB3 = vtB.rearrange("p (c f) -> p c f", c=nB)
            nc.vector.memset(vtB3[:, :, hd : hd + 1], 1.0)
            for ck in range(nA):
                nc.sync.dma_start(
                    out=vtA3[:, ck, 0:hd],
                    in_=vtT_A[:, ck * 128 : (ck + 1) * 128],
                    transpose=True,
                )
                if ck % 2 == 1:
                    ckb = ck // 2
                    nc.sync.dma_start(
                        out=vtB3[:, ckb, 0:hd],
                        in_=vtT_B[64:128, ckb * 128 : (ckb + 1) * 128],
                        transpose=True,
                    )
            blk_state[bi] = (kt_A, kt_B, vtA, vtB, nA, nB)

        def phase_b(bi):
            b, p, ib = blocks[bi]
            kt_A, kt_B, vtA, vtB, nA, nB = blk_state.pop(bi)
            VP = hd + 16
            if (b, p) not in avp_live:
                avp_live[(b, p)] = {
                    (h, nt): av_ps.tile(
                        [hd + 1, tt], F32, tag=f"av{h}{nt}", name="av_psum"
                    )
                    for h in range(2)
                    for nt in range(ntt)
                }
            avp = avp_live[(b, p)]

            for ck in range(nA):
                do_B = ck % 2 == 1
                ckb = ck // 2
                ptA = ppool.tile([128, Tl], FR, tag="pA", name="ptA")
                sa = sc_ps.tile([128, 2 * tt], F32, tag="sc", name="sc_psum")
                if do_B:
                    ptB = ppool.tile([128, Tl], FR, tag="pB", name="ptB")
                    sb_ = kv_ps.tile([128, 2 * tt], F32, tag="kv", name="kv_psum")
                for nt in range(ntt):
                    _mm(
                        nc,
                        sa[:, nt * tt : (nt + 1) * tt],
                        kt_A[:, ck * 128 : (ck + 1) * 128],
                        qt_sb[(b, p)][0:64, nt * tt : (nt + 1) * tt],
                        start=True,
                        stop=True,
                    )
                    if do_B:
                        _mm(
                            nc,
                            sb_[:, nt * tt : (nt + 1) * tt],
                            kt_B[64:128, ckb * 128 : (ckb + 1) * 128],
                            qt_sb[(b, p)][64:128, nt * tt : (nt + 1) * tt],
                            start=True,
                            stop=True,
                        )
                nc.scalar.activation(ptA, sa, AF.Exp, scale=scale)
                if do_B:
                    nc.scalar.activation(ptB, sb_, AF.Exp, scale=scale)
                flush_pending()
                pending.append(
                    (
                        0,
                        avp,
                        vtA[:, ck * VP : ck * VP + hd + 1],
                        ptA,
                        0,
                        ib == 0 and ck == 0,
                        ib == nblk - 1 and ck == nA - 1,
                    )
                )
                if do_B:
                    pending.append(
                        (
                            0,
                            avp,
                            vtB[:, ckb * VP : ckb * VP + hd + 1],
                            ptB,
                            1,
                            ib == 0 and ckb == 0,
                            ib == nblk - 1 and ckb == nB - 1,
                        )
                    )

            if ib == nblk - 1:
                flush_pending(min_age=0)
                emit_normalize(b, p)
                if p == 1 and b > 0:
                    emit_out_proj(b - 1)  # deferred from previous batch

        nbl = len(blocks)
        phase_a(0)
        if nbl > 1:
            phase_a(1)
        for i in range(nbl):
            phase_b(i)
            if i + 2 < nbl:
                phase_a(i + 2)
        emit_out_proj(nb - 1)

# ---------------------------------------------------------------------------
# Host-side sharding / gathering
# ---------------------------------------------------------------------------


def _core_map():
    """core -> (batches, heads)"""
    m = {}
    for c in range(N_CORES):
        g = c % 4
        bs = [0, 1] if c < 4 else [2, 3]
        hs = [4 * g + i for i in range(4)]
        m[c] = (bs, hs)
    return m


def shard_inputs(inputs, cfg):
    x = np.asarray(inputs["decoder_input"], np.float32)
    e = np.asarray(inputs["encoder_output"], np.float32)
    Wq = np.asarray(inputs["Wq"], np.float32)
    Wk = np.asarray(inputs["Wk"], np.float32)
    Wv = np.asarray(inputs["Wv"], np.float32)
    Wo = np.asarray(inputs["Wo"], np.float32)
    bq = np.asarray(inputs["bq"], np.float32)
    bk = np.asarray(inputs["bk"], np.float32)
    bv = np.asarray(inputs["bv"], np.float32)
    hd = cfg["hd"]
    s4 = cfg["strides"][2]
    in_maps = []
    for c, (bs, hs) in _core_map().items():
        rows = np.concatenate([np.arange(h * hd, (h + 1) * hd) for h in hs])
        xTc = np.ascontiguousarray(
            x[bs].reshape(len(bs) * cfg["T"], cfg["D"]).T.astype(BF16)
        )
        eTc = np.ascontiguousarray(
            e[bs].reshape(len(bs) * cfg["S"], cfg["D"]).T.astype(BF16)
        )
        e4 = e[bs][:, ::s4, :]  # [nb, S//s4, D]
        eT4c = np.ascontiguousarray(
            e4.reshape(len(bs) * (cfg["S"] // s4), cfg["D"]).T.astype(BF16)
        )
        bias = np.stack([bq[rows], bk[rows], bv[rows]]).reshape(6, 128, 1)
        in_maps.append(
            {
                "xT": xTc,
                "eT": eTc,
                "eT4": eT4c,
                "wqT": np.ascontiguousarray(Wq[rows].T.astype(BF16)),
                "wkT": np.ascontiguousarray(Wk[rows].T.astype(BF16)),
                "wvT": np.ascontiguousarray(Wv[rows].T.astype(BF16)),
                "woT": np.ascontiguousarray(Wo[:, rows].T.astype(BF16)),
                "biases": np.ascontiguousarray(bias),
            }
        )
    return in_maps


def gather_output(results, bo, cfg):
    Tl, Dl = cfg["T"], cfg["D"]
    out = np.zeros((B, Tl, Dl), np.float32)
    for c, (bs, _hs) in _core_map().items():
        p = results[c]["partial"].reshape(len(bs), Tl, Dl)
        for i, b in enumerate(bs):
            out[b] += p[i]
    return out + np.asarray(bo, np.float32)[None, None, :]


_COMPILED = None


def _get_compiled():
    global _COMPILED
    if _COMPILED is None:
        _COMPILED = build_program(FULL_CFG)
    return _COMPILED


def run_on_cores(inputs, trace=False, **kw):
    nc = _get_compiled()
    in_maps = shard_inputs(inputs, FULL_CFG)
    res = bass_utils.run_bass_kernel_spmd(
        nc, in_maps, core_ids=list(range(N_CORES)), trace=trace, **kw
    )
    return res


def kernel(**inputs) -> np.ndarray:
    res = run_on_cores(inputs, trace=False)
    return gather_output(res.results, inputs["bo"], FULL_CFG)


# revision 14
# speedup vs baseline: 1.5456x; 1.0892x over previous
# HEPOS cross-attention (strided per-head K/V) on 8 Trainium2 NeuronCores.
#
# Reference computation (per head h, stride s = STRIDE_LIST[h]):
#   Q = x @ Wq.T + bq ; K = e @ Wk.T + bk ; V = e @ Wv.T + bv
#   out_h = softmax(Q_h @ K_h[::s].T / 8) @ V_h[::s]
#   out   = concat_h(out_h) @ Wo.T + bo
#
# Sharding: 64 (batch, head) units over 8 cores. Core c owns head group
# g = c % 4 (heads 4g..4g+3, strides [1,2,4,8]) and batch pair [0,1]
# (c < 4) or [2,3] (c >= 4). Each core computes its heads' contribution
# to out; the host sums the four partials per batch and adds bo.
#
# On-device design (v3):
#  * Heads are processed as two stride PAIRS (sA, 2*sA): (1,2) and (4,8).
#    Head A of a pair lives on SBUF partitions 0-63, head B on 64-127.
#  * Q/K/V projections use the weight matrix as the matmul stationary with
#    both heads packed into the 128 stationary columns (full PE width).
#    K^T/V^T stream the "union" encoder columns (stride sA); head B rows
#    are valid at even union columns and are compacted on evacuation.
#    The stride-4 union for pair (4,8) is pre-packed by the host (eT4).
#  * V^T ([hd, keys]) is flipped to AV orientation ([keys, hd]) with ONE
#    DMA XBAR transpose per (block, head) (3D output access pattern) -
#    zero PE cost, one sync-engine dispatch each.
#  * Scores are computed transposed ([keys, T]); head B's score matmuls
#    use partitions 64-127 (PE row-tile T8) and overlap head A's (T0).
#  * AV accumulates into PSUM tiles resident across all encoder blocks of
#    a (batch, pair); the softmax denominator falls out of a ones-column
#    appended to the V stationary.
#  * Scores of chunk k+1 are issued before AV of chunk k so the PE never
#    waits on the scalar engine's exp.
#  * All DRAM->SBUF loads are single merged DMAs ([128, ndc, *] access
#    patterns); PSUM score tiles are bank-pair wide ([128, 2*tt]) so exp
#    and evacuations run as one instruction per tile.

import os
import sys

import ml_dtypes
import numpy as np

BF16 = ml_dtypes.bfloat16

for _p in ("/opt/trn_rl_repo", "/root/.axon_site/_ro/trn_rl_repo"):
    if os.path.isdir(_p) and _p not in sys.path:
        sys.path.insert(0, _p)

import concourse.bass as bass  # noqa: E402
import concourse.tile as tile  # noqa: E402
from concourse import bacc, mybir  # noqa: E402
from concourse import bass_utils  # noqa: E402

F32 = mybir.dt.float32
MM_DT = mybir.dt.bfloat16  # matmul operand dtype: full PE rate, half DMA
AF = mybir.ActivationFunctionType

D_MODEL = 1024
NUM_HEADS = 16
HEAD_DIM = 64
STRIDE_LIST = [1, 2, 4, 8] * 4
B, T, S = 4, 1024, 4096
N_CORES = 8

FULL_CFG = dict(
    nb=2,  # batches per core
    T=T,
    S=S,
    D=D_MODEL,
    strides=(1, 2, 4, 8),  # per-core head strides; pairs (s0,s1),(s2,s3)
    hd=HEAD_DIM,
    blk=1024,  # encoder S-block (stride-1 columns) per iteration
    tt=512,  # T tile (PSUM free-dim limit for fp32)
)

FR = MM_DT
WHOLE_BLOCK_TRANSPOSE = True


def _mm(nc, out, lhsT, rhs, start, stop):
    nc.tensor.matmul(out, lhsT, rhs, start=start, stop=stop)


def build_program(cfg):
    """Build the per-core Bass/Tile program (same program on all cores)."""
    nb, Tl, Sl, Dl = cfg["nb"], cfg["T"], cfg["S"], cfg["D"]
    strides, hd = cfg["strides"], cfg["hd"]
    assert strides[1] == 2 * strides[0] and strides[3] == 2 * strides[2]
    HP = 4 * hd  # packed head rows (256)
    s4 = strides[2]

    nc = bacc.Bacc(
        "TRN2",
        target_bir_lowering=False,
        debug=False,
        enable_asserts=False,
        num_devices=N_CORES,
    )

    xT = nc.dram_tensor("xT", [Dl, nb * Tl], MM_DT, kind="ExternalInput").ap()
    eT = nc.dram_tensor("eT", [Dl, nb * Sl], MM_DT, kind="ExternalInput").ap()
    eT4 = nc.dram_tensor(
        "eT4", [Dl, nb * (Sl // s4)], MM_DT, kind="ExternalInput"
    ).ap()
    wqT = nc.dram_tensor("wqT", [Dl, HP], MM_DT, kind="ExternalInput").ap()
    wkT = nc.dram_tensor("wkT", [Dl, HP], MM_DT, kind="ExternalInput").ap()
    wvT = nc.dram_tensor("wvT", [Dl, HP], MM_DT, kind="ExternalInput").ap()
    woT = nc.dram_tensor("woT", [HP, Dl], MM_DT, kind="ExternalInput").ap()
    biases = nc.dram_tensor("biases", [6, 128, 1], F32, kind="ExternalInput").ap()
    out = nc.dram_tensor("partial", [nb * Tl, Dl], F32, kind="ExternalOutput").ap()

    with tile.TileContext(nc) as tc:
        _build_tile(tc, cfg, xT, eT, eT4, wqT, wkT, wvT, woT, biases, out)

    nc.compile()
    return nc


def _build_tile(tc, cfg, xT, eT, eT4, wqT, wkT, wvT, woT, biases, out):
    nc = tc.nc
    nb, Tl, Sl, Dl = cfg["nb"], cfg["T"], cfg["S"], cfg["D"]
    strides, hd = cfg["strides"], cfg["hd"]
    blk, tt = cfg["blk"], cfg["tt"]
    ndc = Dl // 128
    nblk = Sl // blk
    ntt = Tl // tt
    assert ntt == 2, "wide PSUM tiles assume T == 2*tt"
    HP = 4 * hd
    scale = 1.0 / float(np.sqrt(hd))

    from contextlib import ExitStack

    with ExitStack() as ctx:
        wpool = ctx.enter_context(tc.tile_pool(name="weights", bufs=1))
        qtpool = ctx.enter_context(tc.tile_pool(name="qt", bufs=1))
        etpool = ctx.enter_context(tc.tile_pool(name="et", bufs=3))
        ktpool = ctx.enter_context(tc.tile_pool(name="kt", bufs=3))
        vtpool = ctx.enter_context(tc.tile_pool(name="vtT", bufs=2))
        vpool = ctx.enter_context(tc.tile_pool(name="v", bufs=3))
        ppool = ctx.enter_context(tc.tile_pool(name="p", bufs=4))
        npool = ctx.enter_context(tc.tile_pool(name="norm", bufs=2))
        otpool = ctx.enter_context(tc.tile_pool(name="ot", bufs=2))
        obpool = ctx.enter_context(tc.tile_pool(name="outs", bufs=3))
        # PSUM: sc/kv are bank-pair wide ([128, 2*tt] fp32 = 2 banks each),
        # av holds 4 single-bank accumulators -> 8 banks total.
        sc_ps = ctx.enter_context(tc.tile_pool(name="sc_ps", bufs=1, space="PSUM"))
        kv_ps = ctx.enter_context(tc.tile_pool(name="kv_ps", bufs=1, space="PSUM"))
        av_ps = ctx.enter_context(tc.tile_pool(name="av_ps", bufs=1, space="PSUM"))

        # ---- weights into SBUF (one DMA per tensor) ----
        wq_sb = wpool.tile([128, ndc * HP], FR, tag="wq", name="wq_sb")
        wk_sb = wpool.tile([128, ndc * HP], FR, tag="wk", name="wk_sb")
        wv_sb = wpool.tile([128, ndc * HP], FR, tag="wv", name="wv_sb")
        wo_sb = wpool.tile([128, 2 * Dl], FR, tag="wo", name="wo_sb")
        bias_sb = wpool.tile([128, 6], F32, tag="bias", name="bias_sb")
        ones_sb = wpool.tile([128, 1], F32, tag="ones", name="ones_sb")

        def wslice(wsb, dc, p):
            return wsb[:, dc * HP + p * 128 : dc * HP + (p + 1) * 128]

        # encoder block list + DMA helper (defined early so the first
        # block's load can be interleaved with the weight loads).
        # Blocks are uniform in UNION columns (blk per block) so pair (4,8)
        # gets one full-sized block instead of four tiny ones.
        def block_params(p):
            sA = strides[2 * p]
            Scols = Sl // sA
            return dict(
                src=eT if p == 0 else eT4,
                Scols=Scols,
                nblk_p=max(1, Scols // blk),
            )

        blocks = [
            (b, p, ib)
            for b in range(nb)
            for p in range(2)
            for ib in range(block_params(p)["nblk_p"])
        ]

        def block_ublk(p, ib):
            bp = block_params(p)
            return min(blk, bp["Scols"] - ib * blk)

        def emit_et_dma(b, p, ib):
            bp = block_params(p)
            ublk = block_ublk(p, ib)
            et = etpool.tile([128, ndc * ublk], FR, tag="et", name="et_t")
            c0_ = b * bp["Scols"] + ib * blk
            nc.sync.dma_start(
                out=et.rearrange("p (c u) -> p c u", c=ndc),
                in_=bp["src"][:, c0_ : c0_ + ublk].rearrange(
                    "(c p) u -> p c u", p=128
                ),
            )
            return et

        nc.sync.dma_start(
            out=wq_sb.rearrange("p (c h) -> p c h", c=ndc),
            in_=wqT.rearrange("(c p) h -> p c h", p=128),
        )
        xts = {}
        with tc.tile_pool(name="xt", bufs=1) as xpool:
            for b in range(nb):
                xt = xpool.tile([128, ndc * Tl], FR, tag=f"xt{b}", name="xt")
                xts[b] = xt
            nc.sync.dma_start(
                out=xts[0].rearrange("p (c t) -> p c t", c=ndc),
                in_=xT[:, 0:Tl].rearrange("(c p) t -> p c t", p=128),
            )
            nc.sync.dma_start(
                out=wk_sb.rearrange("p (c h) -> p c h", c=ndc),
                in_=wkT.rearrange("(c p) h -> p c h", p=128),
            )
            et_next = emit_et_dma(*blocks[0])
            nc.sync.dma_start(
                out=wv_sb.rearrange("p (c h) -> p c h", c=ndc),
                in_=wvT.rearrange("(c p) h -> p c h", p=128),
            )
            for b in range(1, nb):
                nc.sync.dma_start(
                    out=xts[b].rearrange("p (c t) -> p c t", c=ndc),
                    in_=xT[:, b * Tl : (b + 1) * Tl].rearrange(
                        "(c p) t -> p c t", p=128
                    ),
                )
            nc.sync.dma_start(
                out=wo_sb.rearrange("p (g d) -> p g d", g=2),
                in_=woT.rearrange("(g p) d -> p g d", p=128),
            )
            nc.sync.dma_start(
                out=bias_sb, in_=biases.rearrange("g p one -> p (g one)")
            )
            nc.vector.memset(ones_sb, 1.0)

            # ---- phase 1: Q^T = (x @ Wq.T + bq)^T, head pairs on partitions
            qt_sb = {}  # (b, pair) -> [128, T] tile
            for b in range(nb):
                for p in range(2):
                    qt = qtpool.tile([128, Tl], FR, tag=f"qt{b}{p}", name="qt")
                    qt_sb[(b, p)] = qt
                    ps = sc_ps.tile([128, 2 * tt], F32, tag="sc", name="sc_psum")
                    for nt in range(ntt):
                        for dc in range(ndc):
                            _mm(
                                nc,
                                ps[:, nt * tt : (nt + 1) * tt],
                                wslice(wq_sb, dc, p),
                                xts[b][:, dc * Tl + nt * tt : dc * Tl + (nt + 1) * tt],
                                start=(dc == 0),
                                stop=(dc == ndc - 1),
                            )
                    nc.scalar.activation(
                        qt, ps, AF.Identity, bias=bias_sb[:, p : p + 1]
                    )

        # ---- phase 2: attention per (batch, pair), out proj per batch ----
        # The per-block work is split into phase A (K^T/V^T projection,
        # evacuation, V transposes, next-block encoder DMA) and phase B
        # (scores/exp/AV chunk loop), software-pipelined one block deep:
        #   pA(0) pA(1) pB(0) pA(2) pB(1) ... pA(n-1) pB(n-3) pB(n-2) pB(n-1)
        # so V transposes are dispatched a full block before their AV
        # consumes them and the normalize chain never blocks evacuations.
        # AV emission inside phase B additionally lags scores by two chunk
        # steps so the PE never waits on the scalar engine's exp.
        assert Dl <= 2 * tt
        ot_sb = {}
        avp_live = {}
        blk_state = {}
        pending = []  # (age, avp, vt, pt, h, first, last)

        def flush_pending(min_age=2):
            keep = []
            for age, avp, vt, pt, h, first, last in pending:
                if age >= min_age:
                    for nt in range(ntt):
                        _mm(
                            nc,
                            avp[(h, nt)],
                            vt,
                            pt[:, nt * tt : (nt + 1) * tt],
                            start=first,
                            stop=last,
                        )
                else:
                    keep.append((age + 1, avp, vt, pt, h, first, last))
            pending[:] = keep

        def emit_normalize(b, p):
            """avp PSUM -> normalized ot (rows 0-63 head A, 64-127 head B)."""
            avp = avp_live.pop((b, p))
            ot = otpool.tile([128, Tl], FR, tag=f"ot{p}", name="ot")
            ot_sb[(b, p)] = ot
            for nt in range(ntt):
                for h in range(2):
                    r0 = npool.tile([1, tt], F32, tag="r0", name="r0")
                    nc.vector.reciprocal(r0, avp[(h, nt)][hd : hd + 1, :])
                    rb = npool.tile([hd, tt], F32, tag="rb", name="rbcast")
                    nc.gpsimd.partition_broadcast(rb, r0)
                    nc.vector.tensor_mul(
                        ot[h * hd : (h + 1) * hd, nt * tt : (nt + 1) * tt],
                        avp[(h, nt)][0:hd, :],
                        rb,
                    )

        def emit_out_proj(b):
            for tc_i in range(Tl // 128):
                ops = sc_ps.tile([128, 2 * tt], F32, tag="sc", name="sc_psum")
                for j in range(0, Dl, tt):
                    dw = min(tt, Dl - j)
                    for p in range(2):
                        _mm(
                            nc,
                            ops[:, j : j + dw],
                            ot_sb[(b, p)][:, tc_i * 128 : (tc_i + 1) * 128],
                            wo_sb[:, p * Dl + j : p * Dl + j + dw],
                            start=(p == 0),
                            stop=(p == 1),
                        )
                ob = obpool.tile([128, Dl], F32, tag="ob", name="ob")
                nc.vector.tensor_copy(ob, ops[:, 0:Dl])
                nc.sync.dma_start(
                    out=out[b * Tl + tc_i * 128 : b * Tl + (tc_i + 1) * 128, :],
                    in_=ob,
                )

        ets = {0: et_next}

        def phase_a(bi):
            b, p, ib = blocks[bi]
            ublk = block_ublk(p, ib)
            nA = ublk // 128
            nB = nA // 2
            et = ets.pop(bi)
            # K^T proj (packed pair; B compacted to even union cols)
            kt_A = ktpool.tile([64, ublk], FR, tag="ktA", name="kt_A")
            kt_B = ktpool.tile([128, ublk // 2], FR, tag="ktB", name="kt_B")
            vtT_A = vtpool.tile([64, ublk], FR, tag="vtA", name="vtT_A")
            vtT_B = vtpool.tile([128, ublk // 2], FR, tag="vtB", name="vtT_B")
            kps = sc_ps.tile([128, 2 * tt], F32, tag="sc", name="sc_psum")
            for c0 in range(0, ublk, tt):
                cw = min(tt, ublk - c0)
                for dc in range(ndc):
                    _mm(
                        nc,
                        kps[:, c0 : c0 + cw],
                        wslice(wk_sb, dc, p),
                        et[:, dc * ublk + c0 : dc * ublk + c0 + cw],
                        start=(dc == 0),
                        stop=(dc == ndc - 1),
                    )
            nc.vector.tensor_scalar_add(
                kt_A, kps[0:64, 0:ublk], bias_sb[0:64, 2 + p : 3 + p]
            )
            nc.vector.tensor_scalar_add(
                kt_B[64:128, :],
                kps[64:128, 0:ublk:2],
                bias_sb[64:128, 2 + p : 3 + p],
            )
            # V^T proj (same streaming, wv stationary)
            vps = kv_ps.tile([128, 2 * tt], F32, tag="kv", name="kv_psum")
            for c0 in range(0, ublk, tt):
                cw = min(tt, ublk - c0)
                for dc in range(ndc):
                    _mm(
                        nc,
                        vps[:, c0 : c0 + cw],
                        wslice(wv_sb, dc, p),
                        et[:, dc * ublk + c0 : dc * ublk + c0 + cw],
                        start=(dc == 0),
                        stop=(dc == ndc - 1),
                    )
            nc.vector.tensor_scalar_add(
                vtT_A, vps[0:64, 0:ublk], bias_sb[0:64, 4 + p : 5 + p]
            )
            nc.vector.tensor_scalar_add(
                vtT_B[64:128, :],
                vps[64:128, 0:ublk:2],
                bias_sb[64:128, 4 + p : 5 + p],
            )

            # prefetch the NEXT block's encoder tile before the transposes
            if bi + 1 < len(blocks):
                ets[bi + 1] = emit_et_dma(*blocks[bi + 1])

            # V -> [keys, hd] via per-chunk DMA XBAR transposes, dispatched
            # in consumption order (A0 A1 B0 A2 A3 B1 ...). Chunk pitch 80
            # elems (160B) keeps destinations 32B-aligned (xbar encoding).
            VP = hd + 16
            vtA = vpool.tile([128, nA * VP], FR, tag="vA", name="vtA")
            vtA3 = vtA.rearrange("p (c f) -> p c f", c=nA)
            nc.vector.memset(vtA3[:, :, hd : hd + 1], 1.0)
            vtB = vpool.tile([128, nB * VP], FR, tag="vB", name="vtB")
            vtB3 = vtB.rearrange("p (c f) -> p c f", c=nB)
            nc.vector.memset(vtB3[:, :, hd : hd + 1], 1.0)
            if WHOLE_BLOCK_TRANSPOSE:
                nc.sync.dma_start(out=vtA3[:, :, 0:hd], in_=vtT_A, transpose=True)
                nc.sync.dma_start(
                    out=vtB3[:, :, 0:hd], in_=vtT_B[64:128, :], transpose=True
                )
            else:
                for ck in range(nA):
                    nc.sync.dma_start(
                        out=vtA3[:, ck, 0:hd],
                        in_=vtT_A[:, ck * 128 : (ck + 1) * 128],
                        transpose=True,
                    )
                    if ck % 2 == 1:
                        ckb = ck // 2
                        nc.sync.dma_start(
                            out=vtB3[:, ckb, 0:hd],
                            in_=vtT_B[64:128, ckb * 128 : (ckb + 1) * 128],
                            transpose=True,
                        )
            blk_state[bi] = (kt_A, kt_B, vtA, vtB, nA, nB)

        def phase_b(bi):
            b, p, ib = blocks[bi]
            kt_A, kt_B, vtA, vtB, nA, nB = blk_state.pop(bi)
            nblk_p = block_params(p)["nblk_p"]
            VP = hd + 16
            if (b, p) not in avp_live:
                avp_live[(b, p)] = {
                    (h, nt): av_ps.tile(
                        [hd + 1, tt], F32, tag=f"av{h}{nt}", name="av_psum"
                    )
                    for h in range(2)
                    for nt in range(ntt)
                }
            avp = avp_live[(b, p)]

            for ck in range(nA):
                do_B = ck % 2 == 1
                ckb = ck // 2
                ptA = ppool.tile([128, Tl], FR, tag="pA", name="ptA")
                sa = sc_ps.tile([128, 2 * tt], F32, tag="sc", name="sc_psum")
                if do_B:
                    ptB = ppool.tile([128, Tl], FR, tag="pB", name="ptB")
                    sb_ = kv_ps.tile([128, 2 * tt], F32, tag="kv", name="kv_psum")
                for nt in range(ntt):
                    _mm(
                        nc,
                        sa[:, nt * tt : (nt + 1) * tt],
                        kt_A[:, ck * 128 : (ck + 1) * 128],
                        qt_sb[(b, p)][0:64, nt * tt : (nt + 1) * tt],
                        start=True,
                        stop=True,
                    )
                    if do_B:
                        _mm(
                            nc,
                            sb_[:, nt * tt : (nt + 1) * tt],
                            kt_B[64:128, ckb * 128 : (ckb + 1) * 128],
                            qt_sb[(b, p)][64:128, nt * tt : (nt + 1) * tt],
                            start=True,
                            stop=True,
                        )
                nc.scalar.activation(ptA, sa, AF.Exp, scale=scale)
                if do_B:
                    nc.scalar.activation(ptB, sb_, AF.Exp, scale=scale)
                flush_pending()
                pending.append(
                    (
                        0,
                        avp,
                        vtA[:, ck * VP : ck * VP + hd + 1],
                        ptA,
                        0,
                        ib == 0 and ck == 0,
                        ib == nblk_p - 1 and ck == nA - 1,
                    )
                )
                if do_B:
                    pending.append(
                        (
                            0,
                            avp,
                            vtB[:, ckb * VP : ckb * VP + hd + 1],
                            ptB,
                            1,
                            ib == 0 and ckb == 0,
                            ib == nblk_p - 1 and ckb == nB - 1,
                        )
                    )

            if ib == nblk_p - 1:
                flush_pending(min_age=0)
                emit_normalize(b, p)
                if p == 1 and b > 0:
                    emit_out_proj(b - 1)  # deferred from previous batch

        nbl = len(blocks)
        phase_a(0)
        if nbl > 1:
            phase_a(1)
        for i in range(nbl):
            phase_b(i)
            if i + 2 < nbl:
                phase_a(i + 2)
        emit_out_proj(nb - 1)

# ---------------------------------------------------------------------------
# Host-side sharding / gathering
# ---------------------------------------------------------------------------


def _core_map():
    """core -> (batches, heads)"""
    m = {}
    for c in range(N_CORES):
        g = c % 4
        bs = [0, 1] if c < 4 else [2, 3]
        hs = [4 * g + i for i in range(4)]
        m[c] = (bs, hs)
    return m


def shard_inputs(inputs, cfg):
    x = np.asarray(inputs["decoder_input"], np.float32)
    e = np.asarray(inputs["encoder_output"], np.float32)
    Wq = np.asarray(inputs["Wq"], np.float32)
    Wk = np.asarray(inputs["Wk"], np.float32)
    Wv = np.asarray(inputs["Wv"], np.float32)
    Wo = np.asarray(inputs["Wo"], np.float32)
    bq = np.asarray(inputs["bq"], np.float32)
    bk = np.asarray(inputs["bk"], np.float32)
    bv = np.asarray(inputs["bv"], np.float32)
    hd = cfg["hd"]
    s4 = cfg["strides"][2]
    in_maps = []
    for c, (bs, hs) in _core_map().items():
        rows = np.concatenate([np.arange(h * hd, (h + 1) * hd) for h in hs])
        xTc = np.ascontiguousarray(
            x[bs].reshape(len(bs) * cfg["T"], cfg["D"]).T.astype(BF16)
        )
        eTc = np.ascontiguousarray(
            e[bs].reshape(len(bs) * cfg["S"], cfg["D"]).T.astype(BF16)
        )
        e4 = e[bs][:, ::s4, :]  # [nb, S//s4, D]
        eT4c = np.ascontiguousarray(
            e4.reshape(len(bs) * (cfg["S"] // s4), cfg["D"]).T.astype(BF16)
        )
        bias = np.stack([bq[rows], bk[rows], bv[rows]]).reshape(6, 128, 1)
        in_maps.append(
            {
                "xT": xTc,
                "eT": eTc,
                "eT4": eT4c,
                "wqT": np.ascontiguousarray(Wq[rows].T.astype(BF16)),
                "wkT": np.ascontiguousarray(Wk[rows].T.astype(BF16)),
                "wvT": np.ascontiguousarray(Wv[rows].T.astype(BF16)),
                "woT": np.ascontiguousarray(Wo[:, rows].T.astype(BF16)),
                "biases": np.ascontiguousarray(bias),
            }
        )
    return in_maps


def gather_output(results, bo, cfg):
    Tl, Dl = cfg["T"], cfg["D"]
    out = np.zeros((B, Tl, Dl), np.float32)
    for c, (bs, _hs) in _core_map().items():
        p = results[c]["partial"].reshape(len(bs), Tl, Dl)
        for i, b in enumerate(bs):
            out[b] += p[i]
    return out + np.asarray(bo, np.float32)[None, None, :]


_COMPILED = None


def _get_compiled():
    global _COMPILED
    if _COMPILED is None:
        _COMPILED = build_program(FULL_CFG)
    return _COMPILED


def run_on_cores(inputs, trace=False, **kw):
    nc = _get_compiled()
    in_maps = shard_inputs(inputs, FULL_CFG)
    res = bass_utils.run_bass_kernel_spmd(
        nc, in_maps, core_ids=list(range(N_CORES)), trace=trace, **kw
    )
    return res


def kernel(**inputs) -> np.ndarray:
    res = run_on_cores(inputs, trace=False)
    return gather_output(res.results, inputs["bo"], FULL_CFG)


# revision 15
# speedup vs baseline: 1.5795x; 1.0219x over previous
# HEPOS cross-attention (strided per-head K/V) on 8 Trainium2 NeuronCores.
#
# Reference computation (per head h, stride s = STRIDE_LIST[h]):
#   Q = x @ Wq.T + bq ; K = e @ Wk.T + bk ; V = e @ Wv.T + bv
#   out_h = softmax(Q_h @ K_h[::s].T / 8) @ V_h[::s]
#   out   = concat_h(out_h) @ Wo.T + bo
#
# Sharding: 64 (batch, head) units over 8 cores. Core c owns head group
# g = c % 4 (heads 4g..4g+3, strides [1,2,4,8]) and batch pair [0,1]
# (c < 4) or [2,3] (c >= 4). Each core computes its heads' contribution
# to out; the host sums the four partials per batch and adds bo.
#
# On-device design (v3):
#  * Heads are processed as two stride PAIRS (sA, 2*sA): (1,2) and (4,8).
#    Head A of a pair lives on SBUF partitions 0-63, head B on 64-127.
#  * Q/K/V projections use the weight matrix as the matmul stationary with
#    both heads packed into the 128 stationary columns (full PE width).
#    K^T/V^T stream the "union" encoder columns (stride sA); head B rows
#    are valid at even union columns and are compacted on evacuation.
#    The stride-4 union for pair (4,8) is pre-packed by the host (eT4).
#  * V^T ([hd, keys]) is flipped to AV orientation ([keys, hd]) with ONE
#    DMA XBAR transpose per (block, head) (3D output access pattern) -
#    zero PE cost, one sync-engine dispatch each.
#  * Scores are computed transposed ([keys, T]); head B's score matmuls
#    use partitions 64-127 (PE row-tile T8) and overlap head A's (T0).
#  * AV accumulates into PSUM tiles resident across all encoder blocks of
#    a (batch, pair); the softmax denominator falls out of a ones-column
#    appended to the V stationary.
#  * Scores of chunk k+1 are issued before AV of chunk k so the PE never
#    waits on the scalar engine's exp.
#  * All DRAM->SBUF loads are single merged DMAs ([128, ndc, *] access
#    patterns); PSUM score tiles are bank-pair wide ([128, 2*tt]) so exp
#    and evacuations run as one instruction per tile.

import os
import sys

import ml_dtypes
import numpy as np

BF16 = ml_dtypes.bfloat16

for _p in ("/opt/trn_rl_repo", "/root/.axon_site/_ro/trn_rl_repo"):
    if os.path.isdir(_p) and _p not in sys.path:
        sys.path.insert(0, _p)

import concourse.bass as bass  # noqa: E402
import concourse.tile as tile  # noqa: E402
from concourse import bacc, mybir  # noqa: E402
from concourse import bass_utils  # noqa: E402

F32 = mybir.dt.float32
MM_DT = mybir.dt.bfloat16  # matmul operand dtype: full PE rate, half DMA
AF = mybir.ActivationFunctionType

D_MODEL = 1024
NUM_HEADS = 16
HEAD_DIM = 64
STRIDE_LIST = [1, 2, 4, 8] * 4
B, T, S = 4, 1024, 4096
N_CORES = 8

FULL_CFG = dict(
    nb=2,  # batches per core
    T=T,
    S=S,
    D=D_MODEL,
    strides=(1, 2, 4, 8),  # per-core head strides; pairs (s0,s1),(s2,s3)
    hd=HEAD_DIM,
    blk=1024,  # encoder S-block (stride-1 columns) per iteration
    tt=512,  # T tile (PSUM free-dim limit for fp32)
)

FR = MM_DT
WHOLE_BLOCK_TRANSPOSE = True


def _mm(nc, out, lhsT, rhs, start, stop):
    nc.tensor.matmul(out, lhsT, rhs, start=start, stop=stop)


def build_program(cfg):
    """Build the per-core Bass/Tile program (same program on all cores)."""
    nb, Tl, Sl, Dl = cfg["nb"], cfg["T"], cfg["S"], cfg["D"]
    strides, hd = cfg["strides"], cfg["hd"]
    assert strides[1] == 2 * strides[0] and strides[3] == 2 * strides[2]
    HP = 4 * hd  # packed head rows (256)
    s4 = strides[2]

    nc = bacc.Bacc(
        "TRN2",
        target_bir_lowering=False,
        debug=False,
        enable_asserts=False,
        num_devices=N_CORES,
    )

    xT = nc.dram_tensor("xT", [Dl, nb * Tl], MM_DT, kind="ExternalInput").ap()
    eT = nc.dram_tensor("eT", [Dl, nb * Sl], MM_DT, kind="ExternalInput").ap()
    eT4 = nc.dram_tensor(
        "eT4", [Dl, nb * (Sl // s4)], MM_DT, kind="ExternalInput"
    ).ap()
    wqT = nc.dram_tensor("wqT", [Dl, HP], MM_DT, kind="ExternalInput").ap()
    wkT = nc.dram_tensor("wkT", [Dl, HP], MM_DT, kind="ExternalInput").ap()
    wvT = nc.dram_tensor("wvT", [Dl, HP], MM_DT, kind="ExternalInput").ap()
    woT = nc.dram_tensor("woT", [HP, Dl], MM_DT, kind="ExternalInput").ap()
    biases = nc.dram_tensor("biases", [6, 128, 1], F32, kind="ExternalInput").ap()
    out = nc.dram_tensor("partial", [nb * Tl, Dl], F32, kind="ExternalOutput").ap()

    with tile.TileContext(nc) as tc:
        _build_tile(tc, cfg, xT, eT, eT4, wqT, wkT, wvT, woT, biases, out)

    nc.compile()
    return nc


def _build_tile(tc, cfg, xT, eT, eT4, wqT, wkT, wvT, woT, biases, out):
    nc = tc.nc
    nb, Tl, Sl, Dl = cfg["nb"], cfg["T"], cfg["S"], cfg["D"]
    strides, hd = cfg["strides"], cfg["hd"]
    blk, tt = cfg["blk"], cfg["tt"]
    ndc = Dl // 128
    nblk = Sl // blk
    ntt = Tl // tt
    assert ntt == 2, "wide PSUM tiles assume T == 2*tt"
    HP = 4 * hd
    scale = 1.0 / float(np.sqrt(hd))

    from contextlib import ExitStack

    with ExitStack() as ctx:
        wpool = ctx.enter_context(tc.tile_pool(name="weights", bufs=1))
        qtpool = ctx.enter_context(tc.tile_pool(name="qt", bufs=1))
        etpool = ctx.enter_context(tc.tile_pool(name="et", bufs=3))
        ktpool = ctx.enter_context(tc.tile_pool(name="kt", bufs=3))
        vtpool = ctx.enter_context(tc.tile_pool(name="vtT", bufs=2))
        vpool = ctx.enter_context(tc.tile_pool(name="v", bufs=3))
        ppool = ctx.enter_context(tc.tile_pool(name="p", bufs=4))
        npool = ctx.enter_context(tc.tile_pool(name="norm", bufs=2))
        otpool = ctx.enter_context(tc.tile_pool(name="ot", bufs=2))
        obpool = ctx.enter_context(tc.tile_pool(name="outs", bufs=3))
        # PSUM: sc/kv are bank-pair wide ([128, 2*tt] fp32 = 2 banks each),
        # av holds 4 single-bank accumulators -> 8 banks total.
        sc_ps = ctx.enter_context(tc.tile_pool(name="sc_ps", bufs=1, space="PSUM"))
        kv_ps = ctx.enter_context(tc.tile_pool(name="kv_ps", bufs=1, space="PSUM"))
        av_ps = ctx.enter_context(tc.tile_pool(name="av_ps", bufs=1, space="PSUM"))

        # ---- weights into SBUF (one DMA per tensor) ----
        wq_sb = wpool.tile([128, ndc * HP], FR, tag="wq", name="wq_sb")
        wk_sb = wpool.tile([128, ndc * HP], FR, tag="wk", name="wk_sb")
        wv_sb = wpool.tile([128, ndc * HP], FR, tag="wv", name="wv_sb")
        wo_sb = wpool.tile([128, 2 * Dl], FR, tag="wo", name="wo_sb")
        bias_sb = wpool.tile([128, 6], F32, tag="bias", name="bias_sb")
        ones_sb = wpool.tile([128, 1], F32, tag="ones", name="ones_sb")

        def wslice(wsb, dc, p):
            return wsb[:, dc * HP + p * 128 : dc * HP + (p + 1) * 128]

        # encoder block list + DMA helper (defined early so the first
        # block's load can be interleaved with the weight loads).
        # Blocks are uniform in UNION columns (blk per block) so pair (4,8)
        # gets one full-sized block instead of four tiny ones.
        def block_params(p):
            sA = strides[2 * p]
            Scols = Sl // sA
            return dict(
                src=eT if p == 0 else eT4,
                Scols=Scols,
                nblk_p=max(1, Scols // blk),
            )

        blocks = [
            (b, p, ib)
            for b in range(nb)
            for p in range(2)
            for ib in range(block_params(p)["nblk_p"])
        ]

        def block_ublk(p, ib):
            bp = block_params(p)
            return min(blk, bp["Scols"] - ib * blk)

        def emit_et_dma(b, p, ib):
            bp = block_params(p)
            ublk = block_ublk(p, ib)
            et = etpool.tile([128, ndc * ublk], FR, tag="et", name="et_t")
            c0_ = b * bp["Scols"] + ib * blk
            nc.sync.dma_start(
                out=et.rearrange("p (c u) -> p c u", c=ndc),
                in_=bp["src"][:, c0_ : c0_ + ublk].rearrange(
                    "(c p) u -> p c u", p=128
                ),
            )
            return et

        nc.sync.dma_start(
            out=wq_sb.rearrange("p (c h) -> p c h", c=ndc),
            in_=wqT.rearrange("(c p) h -> p c h", p=128),
        )
        xts = {}
        with tc.tile_pool(name="xt", bufs=1) as xpool:
            for b in range(nb):
                xt = xpool.tile([128, ndc * Tl], FR, tag=f"xt{b}", name="xt")
                xts[b] = xt
            nc.sync.dma_start(
                out=xts[0].rearrange("p (c t) -> p c t", c=ndc),
                in_=xT[:, 0:Tl].rearrange("(c p) t -> p c t", p=128),
            )
            nc.sync.dma_start(
                out=wk_sb.rearrange("p (c h) -> p c h", c=ndc),
                in_=wkT.rearrange("(c p) h -> p c h", p=128),
            )
            et_next = emit_et_dma(*blocks[0])
            nc.sync.dma_start(
                out=wv_sb.rearrange("p (c h) -> p c h", c=ndc),
                in_=wvT.rearrange("(c p) h -> p c h", p=128),
            )
            for b in range(1, nb):
                nc.sync.dma_start(
                    out=xts[b].rearrange("p (c t) -> p c t", c=ndc),
                    in_=xT[:, b * Tl : (b + 1) * Tl].rearrange(
                        "(c p) t -> p c t", p=128
                    ),
                )
            nc.sync.dma_start(
                out=wo_sb.rearrange("p (g d) -> p g d", g=2),
                in_=woT.rearrange("(g p) d -> p g d", p=128),
            )
            nc.sync.dma_start(
                out=bias_sb, in_=biases.rearrange("g p one -> p (g one)")
            )
            nc.vector.memset(ones_sb, 1.0)

            # ---- phase 1: Q^T = (x @ Wq.T + bq)^T, head pairs on partitions
            qt_sb = {}  # (b, pair) -> [128, T] tile
            for b in range(nb):
                for p in range(2):
                    qt = qtpool.tile([128, Tl], FR, tag=f"qt{b}{p}", name="qt")
                    qt_sb[(b, p)] = qt
                    ps = sc_ps.tile([128, 2 * tt], F32, tag="sc", name="sc_psum")
                    for nt in range(ntt):
                        for dc in range(ndc):
                            _mm(
                                nc,
                                ps[:, nt * tt : (nt + 1) * tt],
                                wslice(wq_sb, dc, p),
                                xts[b][:, dc * Tl + nt * tt : dc * Tl + (nt + 1) * tt],
                                start=(dc == 0),
                                stop=(dc == ndc - 1),
                            )
                    nc.scalar.activation(
                        qt, ps, AF.Identity, bias=bias_sb[:, p : p + 1]
                    )

        # ---- phase 2: attention per (batch, pair), out proj per batch ----
        # The per-block work is split into phase A (K^T/V^T projection,
        # evacuation, V transposes, next-block encoder DMA) and phase B
        # (scores/exp/AV chunk loop), software-pipelined one block deep:
        #   pA(0) pA(1) pB(0) pA(2) pB(1) ... pA(n-1) pB(n-3) pB(n-2) pB(n-1)
        # so V transposes are dispatched a full block before their AV
        # consumes them and the normalize chain never blocks evacuations.
        # AV emission inside phase B additionally lags scores by two chunk
        # steps so the PE never waits on the scalar engine's exp.
        assert Dl <= 2 * tt
        ot_sb = {}
        avp_live = {}
        blk_state = {}
        pending = []  # (age, avp, vt, pt, h, first, last)

        def flush_pending(min_age=2):
            keep = []
            for age, avp, vt, pt, h, first, last in pending:
                if age >= min_age:
                    for nt in range(ntt):
                        _mm(
                            nc,
                            avp[(h, nt)],
                            vt,
                            pt[:, nt * tt : (nt + 1) * tt],
                            start=first,
                            stop=last,
                        )
                else:
                    keep.append((age + 1, avp, vt, pt, h, first, last))
            pending[:] = keep

        def emit_normalize(b, p):
            """avp PSUM -> normalized ot (rows 0-63 head A, 64-127 head B).
            The accumulators are first copied to SBUF with fast scalar-engine
            copies so the PSUM banks free up for the next pair immediately;
            the slow reciprocal chain then runs entirely out of SBUF."""
            avp = avp_live.pop((b, p))
            avc = {}
            for nt in range(ntt):
                for h in range(2):
                    c = npool.tile([hd + 1, tt], F32, tag=f"avc{h}{nt}", name="avc")
                    nc.scalar.copy(c, avp[(h, nt)])
                    avc[(h, nt)] = c
            ot = otpool.tile([128, Tl], FR, tag=f"ot{p}", name="ot")
            ot_sb[(b, p)] = ot
            for nt in range(ntt):
                for h in range(2):
                    r0 = npool.tile([1, tt], F32, tag="r0", name="r0")
                    nc.vector.reciprocal(r0, avc[(h, nt)][hd : hd + 1, :])
                    rb = npool.tile([hd, tt], F32, tag="rb", name="rbcast")
                    nc.gpsimd.partition_broadcast(rb, r0)
                    nc.vector.tensor_mul(
                        ot[h * hd : (h + 1) * hd, nt * tt : (nt + 1) * tt],
                        avc[(h, nt)][0:hd, :],
                        rb,
                    )

        def emit_out_proj(b):
            for tc_i in range(Tl // 128):
                pool, tg = (sc_ps, "sc") if tc_i % 2 == 0 else (kv_ps, "kv")
                ops = pool.tile([128, 2 * tt], F32, tag=tg, name="o_psum")
                for j in range(0, Dl, tt):
                    dw = min(tt, Dl - j)
                    for p in range(2):
                        _mm(
                            nc,
                            ops[:, j : j + dw],
                            ot_sb[(b, p)][:, tc_i * 128 : (tc_i + 1) * 128],
                            wo_sb[:, p * Dl + j : p * Dl + j + dw],
                            start=(p == 0),
                            stop=(p == 1),
                        )
                ob = obpool.tile([128, Dl], F32, tag="ob", name="ob")
                nc.vector.tensor_copy(ob, ops[:, 0:Dl])
                nc.sync.dma_start(
                    out=out[b * Tl + tc_i * 128 : b * Tl + (tc_i + 1) * 128, :],
                    in_=ob,
                )

        ets = {0: et_next}

        def phase_a(bi):
            b, p, ib = blocks[bi]
            ublk = block_ublk(p, ib)
            nA = ublk // 128
            nB = nA // 2
            et = ets.pop(bi)
            # K^T proj (packed pair; B compacted to even union cols)
            kt_A = ktpool.tile([64, ublk], FR, tag="ktA", name="kt_A")
            kt_B = ktpool.tile([128, ublk // 2], FR, tag="ktB", name="kt_B")
            vtT_A = vtpool.tile([64, ublk], FR, tag="vtA", name="vtT_A")
            vtT_B = vtpool.tile([128, ublk // 2], FR, tag="vtB", name="vtT_B")
            kps = sc_ps.tile([128, 2 * tt], F32, tag="sc", name="sc_psum")
            for c0 in range(0, ublk, tt):
                cw = min(tt, ublk - c0)
                for dc in range(ndc):
                    _mm(
                        nc,
                        kps[:, c0 : c0 + cw],
                        wslice(wk_sb, dc, p),
                        et[:, dc * ublk + c0 : dc * ublk + c0 + cw],
                        start=(dc == 0),
                        stop=(dc == ndc - 1),
                    )
            nc.vector.tensor_scalar_add(
                kt_A, kps[0:64, 0:ublk], bias_sb[0:64, 2 + p : 3 + p]
            )
            nc.vector.tensor_scalar_add(
                kt_B[64:128, :],
                kps[64:128, 0:ublk:2],
                bias_sb[64:128, 2 + p : 3 + p],
            )
            # V^T proj (same streaming, wv stationary)
            vps = kv_ps.tile([128, 2 * tt], F32, tag="kv", name="kv_psum")
            for c0 in range(0, ublk, tt):
                cw = min(tt, ublk - c0)
                for dc in range(ndc):
                    _mm(
                        nc,
                        vps[:, c0 : c0 + cw],
                        wslice(wv_sb, dc, p),
                        et[:, dc * ublk + c0 : dc * ublk + c0 + cw],
                        start=(dc == 0),
                        stop=(dc == ndc - 1),
                    )
            nc.vector.tensor_scalar_add(
                vtT_A, vps[0:64, 0:ublk], bias_sb[0:64, 4 + p : 5 + p]
            )
            nc.vector.tensor_scalar_add(
                vtT_B[64:128, :],
                vps[64:128, 0:ublk:2],
                bias_sb[64:128, 4 + p : 5 + p],
            )

            # prefetch the NEXT block's encoder tile before the transposes
            if bi + 1 < len(blocks):
                ets[bi + 1] = emit_et_dma(*blocks[bi + 1])

            # V -> [keys, hd] via per-chunk DMA XBAR transposes, dispatched
            # in consumption order (A0 A1 B0 A2 A3 B1 ...). Chunk pitch 80
            # elems (160B) keeps destinations 32B-aligned (xbar encoding).
            VP = hd + 16
            vtA = vpool.tile([128, nA * VP], FR, tag="vA", name="vtA")
            vtA3 = vtA.rearrange("p (c f) -> p c f", c=nA)
            nc.vector.memset(vtA3[:, :, hd : hd + 1], 1.0)
            vtB = vpool.tile([128, nB * VP], FR, tag="vB", name="vtB")
            vtB3 = vtB.rearrange("p (c f) -> p c f", c=nB)
            nc.vector.memset(vtB3[:, :, hd : hd + 1], 1.0)
            if WHOLE_BLOCK_TRANSPOSE:
                nc.sync.dma_start(out=vtA3[:, :, 0:hd], in_=vtT_A, transpose=True)
                nc.sync.dma_start(
                    out=vtB3[:, :, 0:hd], in_=vtT_B[64:128, :], transpose=True
                )
            else:
                for ck in range(nA):
                    nc.sync.dma_start(
                        out=vtA3[:, ck, 0:hd],
                        in_=vtT_A[:, ck * 128 : (ck + 1) * 128],
                        transpose=True,
                    )
                    if ck % 2 == 1:
                        ckb = ck // 2
                        nc.sync.dma_start(
                            out=vtB3[:, ckb, 0:hd],
                            in_=vtT_B[64:128, ckb * 128 : (ckb + 1) * 128],
                            transpose=True,
                        )
            blk_state[bi] = (kt_A, kt_B, vtA, vtB, nA, nB)

        def phase_b(bi):
            b, p, ib = blocks[bi]
            kt_A, kt_B, vtA, vtB, nA, nB = blk_state.pop(bi)
            nblk_p = block_params(p)["nblk_p"]
            VP = hd + 16
            if (b, p) not in avp_live:
                avp_live[(b, p)] = {
                    (h, nt): av_ps.tile(
                        [hd + 1, tt], F32, tag=f"av{h}{nt}", name="av_psum"
                    )
                    for h in range(2)
                    for nt in range(ntt)
                }
            avp = avp_live[(b, p)]

            for ck in range(nA):
                do_B = ck % 2 == 1
                ckb = ck // 2
                ptA = ppool.tile([128, Tl], FR, tag="pA", name="ptA")
                sa = sc_ps.tile([128, 2 * tt], F32, tag="sc", name="sc_psum")
                if do_B:
                    ptB = ppool.tile([128, Tl], FR, tag="pB", name="ptB")
                    sb_ = kv_ps.tile([128, 2 * tt], F32, tag="kv", name="kv_psum")
                for nt in range(ntt):
                    _mm(
                        nc,
                        sa[:, nt * tt : (nt + 1) * tt],
                        kt_A[:, ck * 128 : (ck + 1) * 128],
                        qt_sb[(b, p)][0:64, nt * tt : (nt + 1) * tt],
                        start=True,
                        stop=True,
                    )
                    if do_B:
                        _mm(
                            nc,
                            sb_[:, nt * tt : (nt + 1) * tt],
                            kt_B[64:128, ckb * 128 : (ckb + 1) * 128],
                            qt_sb[(b, p)][64:128, nt * tt : (nt + 1) * tt],
                            start=True,
                            stop=True,
                        )
                nc.scalar.activation(ptA, sa, AF.Exp, scale=scale)
                if do_B:
                    nc.scalar.activation(ptB, sb_, AF.Exp, scale=scale)
                flush_pending()
                pending.append(
                    (
                        0,
                        avp,
                        vtA[:, ck * VP : ck * VP + hd + 1],
                        ptA,
                        0,
                        ib == 0 and ck == 0,
                        ib == nblk_p - 1 and ck == nA - 1,
                    )
                )
                if do_B:
                    pending.append(
                        (
                            0,
                            avp,
                            vtB[:, ckb * VP : ckb * VP + hd + 1],
                            ptB,
                            1,
                            ib == 0 and ckb == 0,
                            ib == nblk_p - 1 and ckb == nB - 1,
                        )
                    )

            if ib == nblk_p - 1:
                flush_pending(min_age=0)
                emit_normalize(b, p)
            if b > 0 and p == 0 and ib == 0:
                emit_out_proj(b - 1)  # deferred past the previous normalize

        nbl = len(blocks)
        phase_a(0)
        if nbl > 1:
            phase_a(1)
        for i in range(nbl):
            phase_b(i)
            if i + 2 < nbl:
                phase_a(i + 2)
        emit_out_proj(nb - 1)

# ---------------------------------------------------------------------------
# Host-side sharding / gathering
# ---------------------------------------------------------------------------


def _core_map():
    """core -> (batches, heads)"""
    m = {}
    for c in range(N_CORES):
        g = c % 4
        bs = [0, 1] if c < 4 else [2, 3]
        hs = [4 * g + i for i in range(4)]
        m[c] = (bs, hs)
    return m


def shard_inputs(inputs, cfg):
    x = np.asarray(inputs["decoder_input"], np.float32)
    e = np.asarray(inputs["encoder_output"], np.float32)
    Wq = np.asarray(inputs["Wq"], np.float32)
    Wk = np.asarray(inputs["Wk"], np.float32)
    Wv = np.asarray(inputs["Wv"], np.float32)
    Wo = np.asarray(inputs["Wo"], np.float32)
    bq = np.asarray(inputs["bq"], np.float32)
    bk = np.asarray(inputs["bk"], np.float32)
    bv = np.asarray(inputs["bv"], np.float32)
    hd = cfg["hd"]
    s4 = cfg["strides"][2]
    in_maps = []
    for c, (bs, hs) in _core_map().items():
        rows = np.concatenate([np.arange(h * hd, (h + 1) * hd) for h in hs])
        xTc = np.ascontiguousarray(
            x[bs].reshape(len(bs) * cfg["T"], cfg["D"]).T.astype(BF16)
        )
        eTc = np.ascontiguousarray(
            e[bs].reshape(len(bs) * cfg["S"], cfg["D"]).T.astype(BF16)
        )
        e4 = e[bs][:, ::s4, :]  # [nb, S//s4, D]
        eT4c = np.ascontiguousarray(
            e4.reshape(len(bs) * (cfg["S"] // s4), cfg["D"]).T.astype(BF16)
        )
        bias = np.stack([bq[rows], bk[rows], bv[rows]]).reshape(6, 128, 1)
        in_maps.append(
            {
                "xT": xTc,
                "eT": eTc,
                "eT4": eT4c,
                "wqT": np.ascontiguousarray(Wq[rows].T.astype(BF16)),
                "wkT": np.ascontiguousarray(Wk[rows].T.astype(BF16)),
                "wvT": np.ascontiguousarray(Wv[rows].T.astype(BF16)),
                "woT": np.ascontiguousarray(Wo[:, rows].T.astype(BF16)),
                "biases": np.ascontiguousarray(bias),
            }
        )
    return in_maps


def gather_output(results, bo, cfg):
    Tl, Dl = cfg["T"], cfg["D"]
    out = np.zeros((B, Tl, Dl), np.float32)
    for c, (bs, _hs) in _core_map().items():
        p = results[c]["partial"].reshape(len(bs), Tl, Dl)
        for i, b in enumerate(bs):
            out[b] += p[i]
    return out + np.asarray(bo, np.float32)[None, None, :]


_COMPILED = None


def _get_compiled():
    global _COMPILED
    if _COMPILED is None:
        _COMPILED = build_program(FULL_CFG)
    return _COMPILED


def run_on_cores(inputs, trace=False, **kw):
    nc = _get_compiled()
    in_maps = shard_inputs(inputs, FULL_CFG)
    res = bass_utils.run_bass_kernel_spmd(
        nc, in_maps, core_ids=list(range(N_CORES)), trace=trace, **kw
    )
    return res


def kernel(**inputs) -> np.ndarray:
    res = run_on_cores(inputs, trace=False)
    return gather_output(res.results, inputs["bo"], FULL_CFG)


# revision 17
# speedup vs baseline: 1.6968x; 1.0743x over previous
# HEPOS cross-attention (strided per-head K/V) on 8 Trainium2 NeuronCores.
#
# Reference computation (per head h, stride s = STRIDE_LIST[h]):
#   Q = x @ Wq.T + bq ; K = e @ Wk.T + bk ; V = e @ Wv.T + bv
#   out_h = softmax(Q_h @ K_h[::s].T / 8) @ V_h[::s]
#   out   = concat_h(out_h) @ Wo.T + bo
#
# Sharding: 64 (batch, head) units over 8 cores. Core c owns head group
# g = c % 4 (heads 4g..4g+3, strides [1,2,4,8]) and batch pair [0,1]
# (c < 4) or [2,3] (c >= 4). Each core computes its heads' contribution
# to out; the host sums the four partials per batch and adds bo.
#
# On-device design (v3):
#  * Heads are processed as two stride PAIRS (sA, 2*sA): (1,2) and (4,8).
#    Head A of a pair lives on SBUF partitions 0-63, head B on 64-127.
#  * Q/K/V projections use the weight matrix as the matmul stationary with
#    both heads packed into the 128 stationary columns (full PE width).
#    K^T/V^T stream the "union" encoder columns (stride sA); head B rows
#    are valid at even union columns and are compacted on evacuation.
#    The stride-4 union for pair (4,8) is pre-packed by the host (eT4).
#  * V^T ([hd, keys]) is flipped to AV orientation ([keys, hd]) with ONE
#    DMA XBAR transpose per (block, head) (3D output access pattern) -
#    zero PE cost, one sync-engine dispatch each.
#  * Scores are computed transposed ([keys, T]); head B's score matmuls
#    use partitions 64-127 (PE row-tile T8) and overlap head A's (T0).
#  * AV accumulates into PSUM tiles resident across all encoder blocks of
#    a (batch, pair); the softmax denominator falls out of a ones-column
#    appended to the V stationary.
#  * Scores of chunk k+1 are issued before AV of chunk k so the PE never
#    waits on the scalar engine's exp.
#  * All DRAM->SBUF loads are single merged DMAs ([128, ndc, *] access
#    patterns); PSUM score tiles are bank-pair wide ([128, 2*tt]) so exp
#    and evacuations run as one instruction per tile.

import os
import sys

import ml_dtypes
import numpy as np

BF16 = ml_dtypes.bfloat16

for _p in ("/opt/trn_rl_repo", "/root/.axon_site/_ro/trn_rl_repo"):
    if os.path.isdir(_p) and _p not in sys.path:
        sys.path.insert(0, _p)

import concourse.bass as bass  # noqa: E402
import concourse.tile as tile  # noqa: E402
from concourse import bacc, mybir  # noqa: E402
from concourse import bass_utils  # noqa: E402

F32 = mybir.dt.float32
MM_DT = mybir.dt.bfloat16  # matmul operand dtype: full PE rate, half DMA
AF = mybir.ActivationFunctionType

D_MODEL = 1024
NUM_HEADS = 16
HEAD_DIM = 64
STRIDE_LIST = [1, 2, 4, 8] * 4
B, T, S = 4, 1024, 4096
N_CORES = 8

FULL_CFG = dict(
    nb=2,  # batches per core
    T=T,
    S=S,
    D=D_MODEL,
    strides=(1, 2, 4, 8),  # per-core head strides; pairs (s0,s1),(s2,s3)
    hd=HEAD_DIM,
    blk=1024,  # encoder S-block (stride-1 columns) per iteration
    tt=512,  # T tile (PSUM free-dim limit for fp32)
)

FR = MM_DT
WHOLE_BLOCK_TRANSPOSE = True


def _mm(nc, out, lhsT, rhs, start, stop):
    nc.tensor.matmul(out, lhsT, rhs, start=start, stop=stop)


def build_program(cfg):
    """Build the per-core Bass/Tile program (same program on all cores)."""
    nb, Tl, Sl, Dl = cfg["nb"], cfg["T"], cfg["S"], cfg["D"]
    strides, hd = cfg["strides"], cfg["hd"]
    assert strides[1] == 2 * strides[0] and strides[3] == 2 * strides[2]
    HP = 4 * hd  # packed head rows (256)
    s4 = strides[2]

    nc = bacc.Bacc(
        "TRN2",
        target_bir_lowering=False,
        debug=False,
        enable_asserts=False,
        num_devices=N_CORES,
    )

    xT = nc.dram_tensor("xT", [Dl, nb * Tl], MM_DT, kind="ExternalInput").ap()
    eT = nc.dram_tensor("eT", [Dl, nb * Sl], MM_DT, kind="ExternalInput").ap()
    eT4 = nc.dram_tensor(
        "eT4", [Dl, nb * (Sl // s4)], MM_DT, kind="ExternalInput"
    ).ap()
    wqT = nc.dram_tensor("wqT", [Dl, HP], MM_DT, kind="ExternalInput").ap()
    wkT = nc.dram_tensor("wkT", [Dl, HP], MM_DT, kind="ExternalInput").ap()
    wvT = nc.dram_tensor("wvT", [Dl, HP], MM_DT, kind="ExternalInput").ap()
    woT = nc.dram_tensor("woT", [HP, Dl], MM_DT, kind="ExternalInput").ap()
    biases = nc.dram_tensor("biases", [6, 128, 1], F32, kind="ExternalInput").ap()
    out = nc.dram_tensor("partial", [nb * Tl, Dl], F32, kind="ExternalOutput").ap()

    with tile.TileContext(nc) as tc:
        _build_tile(tc, cfg, xT, eT, eT4, wqT, wkT, wvT, woT, biases, out)

    nc.compile()
    return nc


def _build_tile(tc, cfg, xT, eT, eT4, wqT, wkT, wvT, woT, biases, out):
    nc = tc.nc
    nb, Tl, Sl, Dl = cfg["nb"], cfg["T"], cfg["S"], cfg["D"]
    strides, hd = cfg["strides"], cfg["hd"]
    blk, tt = cfg["blk"], cfg["tt"]
    ndc = Dl // 128
    nblk = Sl // blk
    ntt = Tl // tt
    assert ntt == 2, "wide PSUM tiles assume T == 2*tt"
    HP = 4 * hd
    scale = 1.0 / float(np.sqrt(hd))

    from contextlib import ExitStack

    with ExitStack() as ctx:
        wpool = ctx.enter_context(tc.tile_pool(name="weights", bufs=1))
        qtpool = ctx.enter_context(tc.tile_pool(name="qt", bufs=1))
        etpool = ctx.enter_context(tc.tile_pool(name="et", bufs=3))
        ktpool = ctx.enter_context(tc.tile_pool(name="kt", bufs=3))
        vtpool = ctx.enter_context(tc.tile_pool(name="vtT", bufs=2))
        vpool = ctx.enter_context(tc.tile_pool(name="v", bufs=3))
        ppool = ctx.enter_context(tc.tile_pool(name="p", bufs=4))
        npool = ctx.enter_context(tc.tile_pool(name="norm", bufs=2))
        otpool = ctx.enter_context(tc.tile_pool(name="ot", bufs=2))
        obpool = ctx.enter_context(tc.tile_pool(name="outs", bufs=3))
        # PSUM: sc/kv are bank-pair wide ([128, 2*tt] fp32 = 2 banks each),
        # av holds 4 single-bank accumulators -> 8 banks total.
        sc_ps = ctx.enter_context(tc.tile_pool(name="sc_ps", bufs=1, space="PSUM"))
        kv_ps = ctx.enter_context(tc.tile_pool(name="kv_ps", bufs=1, space="PSUM"))
        av_ps = ctx.enter_context(tc.tile_pool(name="av_ps", bufs=1, space="PSUM"))

        # ---- weights into SBUF (one DMA per tensor) ----
        wq_sb = wpool.tile([128, ndc * HP], FR, tag="wq", name="wq_sb")
        wk_sb = wpool.tile([128, ndc * HP], FR, tag="wk", name="wk_sb")
        wv_sb = wpool.tile([128, ndc * HP], FR, tag="wv", name="wv_sb")
        wo_sb = wpool.tile([128, 2 * Dl], FR, tag="wo", name="wo_sb")
        bias_sb = wpool.tile([128, 6], F32, tag="bias", name="bias_sb")
        ones_sb = wpool.tile([128, 1], F32, tag="ones", name="ones_sb")

        def wslice(wsb, dc, p):
            return wsb[:, dc * HP + p * 128 : dc * HP + (p + 1) * 128]

        # encoder block list + DMA helper (defined early so the first
        # block's load can be interleaved with the weight loads).
        # Blocks are uniform in UNION columns (blk per block) so pair (4,8)
        # gets one full-sized block instead of four tiny ones.
        def block_params(p):
            sA = strides[2 * p]
            Scols = Sl // sA
            return dict(
                src=eT if p == 0 else eT4,
                Scols=Scols,
                nblk_p=max(1, Scols // blk),
            )

        blocks = [
            (b, p, ib)
            for b in range(nb)
            for p in range(2)
            for ib in range(block_params(p)["nblk_p"])
        ]

        def block_ublk(p, ib):
            bp = block_params(p)
            return min(blk, bp["Scols"] - ib * blk)

        def emit_et_dma(b, p, ib):
            bp = block_params(p)
            ublk = block_ublk(p, ib)
            et = etpool.tile([128, ndc * ublk], FR, tag="et", name="et_t")
            c0_ = b * bp["Scols"] + ib * blk
            nc.sync.dma_start(
                out=et.rearrange("p (c u) -> p c u", c=ndc),
                in_=bp["src"][:, c0_ : c0_ + ublk].rearrange(
                    "(c p) u -> p c u", p=128
                ),
            )
            return et

        nc.sync.dma_start(
            out=wq_sb.rearrange("p (c h) -> p c h", c=ndc),
            in_=wqT.rearrange("(c p) h -> p c h", p=128),
        )
        xts = {}
        with tc.tile_pool(name="xt", bufs=1) as xpool:
            for b in range(nb):
                xt = xpool.tile([128, ndc * Tl], FR, tag=f"xt{b}", name="xt")
                xts[b] = xt
            nc.sync.dma_start(
                out=xts[0].rearrange("p (c t) -> p c t", c=ndc),
                in_=xT[:, 0:Tl].rearrange("(c p) t -> p c t", p=128),
            )
            nc.sync.dma_start(
                out=wk_sb.rearrange("p (c h) -> p c h", c=ndc),
                in_=wkT.rearrange("(c p) h -> p c h", p=128),
            )
            et_next = emit_et_dma(*blocks[0])
            nc.sync.dma_start(
                out=wv_sb.rearrange("p (c h) -> p c h", c=ndc),
                in_=wvT.rearrange("(c p) h -> p c h", p=128),
            )
            for b in range(1, nb):
                nc.sync.dma_start(
                    out=xts[b].rearrange("p (c t) -> p c t", c=ndc),
                    in_=xT[:, b * Tl : (b + 1) * Tl].rearrange(
                        "(c p) t -> p c t", p=128
                    ),
                )
            nc.sync.dma_start(
                out=wo_sb.rearrange("p (g d) -> p g d", g=2),
                in_=woT.rearrange("(g p) d -> p g d", p=128),
            )
            nc.sync.dma_start(
                out=bias_sb, in_=biases.rearrange("g p one -> p (g one)")
            )
            nc.vector.memset(ones_sb, 1.0)

            # PE warm-up: ~25 dependency-free matmuls on a zeroed tile keep
            # the PE busy while the first DMAs land, so the HAM clock gate
            # opens (1.2 -> 2.4 GHz) before the real matmul stream begins.
            warm = wpool.tile([128, tt], FR, tag="warm", name="warm")
            nc.vector.memset(warm, 0.0)
            wps = kv_ps.tile([128, 2 * tt], F32, tag="kv", name="kv_psum")
            for _ in range(25):
                _mm(nc, wps[:, 0:tt], warm[:, 0:128], warm, start=True, stop=True)

            # ---- phase 1: Q^T = (x @ Wq.T + bq)^T, head pairs on partitions
            qt_sb = {}  # (b, pair) -> [128, T] tile
            for b in range(nb):
                for p in range(2):
                    qt = qtpool.tile([128, Tl], FR, tag=f"qt{b}{p}", name="qt")
                    qt_sb[(b, p)] = qt
                    ps = sc_ps.tile([128, 2 * tt], F32, tag="sc", name="sc_psum")
                    for nt in range(ntt):
                        for dc in range(ndc):
                            _mm(
                                nc,
                                ps[:, nt * tt : (nt + 1) * tt],
                                wslice(wq_sb, dc, p),
                                xts[b][:, dc * Tl + nt * tt : dc * Tl + (nt + 1) * tt],
                                start=(dc == 0),
                                stop=(dc == ndc - 1),
                            )
                    nc.scalar.activation(
                        qt, ps, AF.Identity, bias=bias_sb[:, p : p + 1]
                    )

        # ---- phase 2: attention per (batch, pair), out proj per batch ----
        # The per-block work is split into phase A (K^T/V^T projection,
        # evacuation, V transposes, next-block encoder DMA) and phase B
        # (scores/exp/AV chunk loop), software-pipelined one block deep:
        #   pA(0) pA(1) pB(0) pA(2) pB(1) ... pA(n-1) pB(n-3) pB(n-2) pB(n-1)
        # so V transposes are dispatched a full block before their AV
        # consumes them and the normalize chain never blocks evacuations.
        # AV emission inside phase B additionally lags scores by two chunk
        # steps so the PE never waits on the scalar engine's exp.
        assert Dl <= 2 * tt
        ot_sb = {}
        avp_live = {}
        blk_state = {}
        pending = []  # (age, avp, vt, pt, h, first, last)

        def flush_pending(min_age=2):
            keep = []
            for age, avp, vt, pt, h, first, last in pending:
                if age >= min_age:
                    for nt in range(ntt):
                        _mm(
                            nc,
                            avp[(h, nt)],
                            vt,
                            pt[:, nt * tt : (nt + 1) * tt],
                            start=first,
                            stop=last,
                        )
                else:
                    keep.append((age + 1, avp, vt, pt, h, first, last))
            pending[:] = keep

        def emit_avp_copies(b, p):
            """Copy the AV accumulators PSUM -> SBUF with fast scalar-engine
            copies so the PSUM banks free up for the next pair immediately."""
            avp = avp_live.pop((b, p))
            avc = {}
            for nt in range(ntt):
                for h in range(2):
                    c = npool.tile([hd + 1, tt], F32, tag=f"avc{h}{nt}", name="avc")
                    nc.scalar.copy(c, avp[(h, nt)])
                    avc[(h, nt)] = c
            return avc

        def emit_norm_rest(b, p, avc):
            """Slow reciprocal chain (DVE) out of SBUF; emitted AFTER the
            next phase_a so it never delays pipeline-critical evacuations."""
            ot = otpool.tile([128, Tl], FR, tag=f"ot{p}", name="ot")
            ot_sb[(b, p)] = ot
            for nt in range(ntt):
                for h in range(2):
                    r0 = npool.tile([1, tt], F32, tag="r0", name="r0")
                    nc.vector.reciprocal(r0, avc[(h, nt)][hd : hd + 1, :])
                    rb = npool.tile([hd, tt], F32, tag="rb", name="rbcast")
                    nc.gpsimd.partition_broadcast(rb, r0)
                    nc.vector.tensor_mul(
                        ot[h * hd : (h + 1) * hd, nt * tt : (nt + 1) * tt],
                        avc[(h, nt)][0:hd, :],
                        rb,
                    )

        def emit_out_proj(b):
            for tc_i in range(Tl // 128):
                pool, tg = (sc_ps, "sc") if tc_i % 2 == 0 else (kv_ps, "kv")
                ops = pool.tile([128, 2 * tt], F32, tag=tg, name="o_psum")
                for j in range(0, Dl, tt):
                    dw = min(tt, Dl - j)
                    for p in range(2):
                        _mm(
                            nc,
                            ops[:, j : j + dw],
                            ot_sb[(b, p)][:, tc_i * 128 : (tc_i + 1) * 128],
                            wo_sb[:, p * Dl + j : p * Dl + j + dw],
                            start=(p == 0),
                            stop=(p == 1),
                        )
                ob = obpool.tile([128, Dl], F32, tag="ob", name="ob")
                nc.vector.tensor_copy(ob, ops[:, 0:Dl])
                nc.sync.dma_start(
                    out=out[b * Tl + tc_i * 128 : b * Tl + (tc_i + 1) * 128, :],
                    in_=ob,
                )

        ets = {0: et_next}

        def phase_a(bi):
            b, p, ib = blocks[bi]
            ublk = block_ublk(p, ib)
            nA = ublk // 128
            nB = nA // 2
            et = ets.pop(bi)
            # K^T proj (packed pair; B compacted to even union cols)
            kt_A = ktpool.tile([64, ublk], FR, tag="ktA", name="kt_A")
            kt_B = ktpool.tile([128, ublk // 2], FR, tag="ktB", name="kt_B")
            vtT_A = vtpool.tile([64, ublk], FR, tag="vtA", name="vtT_A")
            vtT_B = vtpool.tile([128, ublk // 2], FR, tag="vtB", name="vtT_B")
            kps = sc_ps.tile([128, 2 * tt], F32, tag="sc", name="sc_psum")
            for c0 in range(0, ublk, tt):
                cw = min(tt, ublk - c0)
                for dc in range(ndc):
                    _mm(
                        nc,
                        kps[:, c0 : c0 + cw],
                        wslice(wk_sb, dc, p),
                        et[:, dc * ublk + c0 : dc * ublk + c0 + cw],
                        start=(dc == 0),
                        stop=(dc == ndc - 1),
                    )
            nc.vector.tensor_scalar_add(
                kt_A, kps[0:64, 0:ublk], bias_sb[0:64, 2 + p : 3 + p]
            )
            nc.vector.tensor_scalar_add(
                kt_B[64:128, :],
                kps[64:128, 0:ublk:2],
                bias_sb[64:128, 2 + p : 3 + p],
            )
            # V^T proj (same streaming, wv stationary)
            vps = kv_ps.tile([128, 2 * tt], F32, tag="kv", name="kv_psum")
            for c0 in range(0, ublk, tt):
                cw = min(tt, ublk - c0)
                for dc in range(ndc):
                    _mm(
                        nc,
                        vps[:, c0 : c0 + cw],
                        wslice(wv_sb, dc, p),
                        et[:, dc * ublk + c0 : dc * ublk + c0 + cw],
                        start=(dc == 0),
                        stop=(dc == ndc - 1),
                    )
            nc.vector.tensor_scalar_add(
                vtT_A, vps[0:64, 0:ublk], bias_sb[0:64, 4 + p : 5 + p]
            )
            nc.vector.tensor_scalar_add(
                vtT_B[64:128, :],
                vps[64:128, 0:ublk:2],
                bias_sb[64:128, 4 + p : 5 + p],
            )

            # prefetch the NEXT block's encoder tile before the transposes
            if bi + 1 < len(blocks):
                ets[bi + 1] = emit_et_dma(*blocks[bi + 1])

            # V -> [keys, hd] via per-chunk DMA XBAR transposes, dispatched
            # in consumption order (A0 A1 B0 A2 A3 B1 ...). Chunk pitch 80
            # elems (160B) keeps destinations 32B-aligned (xbar encoding).
            VP = hd + 16
            vtA = vpool.tile([128, nA * VP], FR, tag="vA", name="vtA")
            vtA3 = vtA.rearrange("p (c f) -> p c f", c=nA)
            nc.vector.memset(vtA3[:, :, hd : hd + 1], 1.0)
            vtB = vpool.tile([128, nB * VP], FR, tag="vB", name="vtB")
            vtB3 = vtB.rearrange("p (c f) -> p c f", c=nB)
            nc.vector.memset(vtB3[:, :, hd : hd + 1], 1.0)
            if WHOLE_BLOCK_TRANSPOSE:
                nc.sync.dma_start(out=vtA3[:, :, 0:hd], in_=vtT_A, transpose=True)
                nc.sync.dma_start(
                    out=vtB3[:, :, 0:hd], in_=vtT_B[64:128, :], transpose=True
                )
            else:
                for ck in range(nA):
                    nc.sync.dma_start(
                        out=vtA3[:, ck, 0:hd],
                        in_=vtT_A[:, ck * 128 : (ck + 1) * 128],
                        transpose=True,
                    )
                    if ck % 2 == 1:
                        ckb = ck // 2
                        nc.sync.dma_start(
                            out=vtB3[:, ckb, 0:hd],
                            in_=vtT_B[64:128, ckb * 128 : (ckb + 1) * 128],
                            transpose=True,
                        )
            blk_state[bi] = (kt_A, kt_B, vtA, vtB, nA, nB)

        def phase_b(bi):
            b, p, ib = blocks[bi]
            kt_A, kt_B, vtA, vtB, nA, nB = blk_state.pop(bi)
            nblk_p = block_params(p)["nblk_p"]
            VP = hd + 16
            if (b, p) not in avp_live:
                avp_live[(b, p)] = {
                    (h, nt): av_ps.tile(
                        [hd + 1, tt], F32, tag=f"av{h}{nt}", name="av_psum"
                    )
                    for h in range(2)
                    for nt in range(ntt)
                }
            avp = avp_live[(b, p)]

            for ck in range(nA):
                do_B = ck % 2 == 1
                ckb = ck // 2
                ptA = ppool.tile([128, Tl], FR, tag="pA", name="ptA")
                sa = sc_ps.tile([128, 2 * tt], F32, tag="sc", name="sc_psum")
                if do_B:
                    ptB = ppool.tile([128, Tl], FR, tag="pB", name="ptB")
                    sb_ = kv_ps.tile([128, 2 * tt], F32, tag="kv", name="kv_psum")
                for nt in range(ntt):
                    _mm(
                        nc,
                        sa[:, nt * tt : (nt + 1) * tt],
                        kt_A[:, ck * 128 : (ck + 1) * 128],
                        qt_sb[(b, p)][0:64, nt * tt : (nt + 1) * tt],
                        start=True,
                        stop=True,
                    )
                    if do_B:
                        _mm(
                            nc,
                            sb_[:, nt * tt : (nt + 1) * tt],
                            kt_B[64:128, ckb * 128 : (ckb + 1) * 128],
                            qt_sb[(b, p)][64:128, nt * tt : (nt + 1) * tt],
                            start=True,
                            stop=True,
                        )
                nc.scalar.activation(ptA, sa, AF.Exp, scale=scale)
                if do_B:
                    nc.scalar.activation(ptB, sb_, AF.Exp, scale=scale)
                flush_pending()
                pending.append(
                    (
                        0,
                        avp,
                        vtA[:, ck * VP : ck * VP + hd + 1],
                        ptA,
                        0,
                        ib == 0 and ck == 0,
                        ib == nblk_p - 1 and ck == nA - 1,
                    )
                )
                if do_B:
                    pending.append(
                        (
                            0,
                            avp,
                            vtB[:, ckb * VP : ckb * VP + hd + 1],
                            ptB,
                            1,
                            ib == 0 and ckb == 0,
                            ib == nblk_p - 1 and ckb == nB - 1,
                        )
                    )

            if ib == nblk_p - 1:
                flush_pending(min_age=0)
                norm_todo.append((b, p, emit_avp_copies(b, p)))
            if b > 0 and p == 0 and ib == 0:
                emit_out_proj(b - 1)  # deferred past the previous normalize

        nbl = len(blocks)
        norm_todo = []
        phase_a(0)
        if nbl > 1:
            phase_a(1)
        for i in range(nbl):
            phase_b(i)
            if i + 2 < nbl:
                phase_a(i + 2)
            while norm_todo:
                nb_, np_, avc_ = norm_todo.pop(0)
                emit_norm_rest(nb_, np_, avc_)
        emit_out_proj(nb - 1)

# ---------------------------------------------------------------------------
# Host-side sharding / gathering
# ---------------------------------------------------------------------------


def _core_map():
    """core -> (batches, heads)"""
    m = {}
    for c in range(N_CORES):
        g = c % 4
        bs = [0, 1] if c < 4 else [2, 3]
        hs = [4 * g + i for i in range(4)]
        m[c] = (bs, hs)
    return m


def shard_inputs(inputs, cfg):
    x = np.asarray(inputs["decoder_input"], np.float32)
    e = np.asarray(inputs["encoder_output"], np.float32)
    Wq = np.asarray(inputs["Wq"], np.float32)
    Wk = np.asarray(inputs["Wk"], np.float32)
    Wv = np.asarray(inputs["Wv"], np.float32)
    Wo = np.asarray(inputs["Wo"], np.float32)
    bq = np.asarray(inputs["bq"], np.float32)
    bk = np.asarray(inputs["bk"], np.float32)
    bv = np.asarray(inputs["bv"], np.float32)
    hd = cfg["hd"]
    s4 = cfg["strides"][2]
    in_maps = []
    for c, (bs, hs) in _core_map().items():
        rows = np.concatenate([np.arange(h * hd, (h + 1) * hd) for h in hs])
        xTc = np.ascontiguousarray(
            x[bs].reshape(len(bs) * cfg["T"], cfg["D"]).T.astype(BF16)
        )
        eTc = np.ascontiguousarray(
            e[bs].reshape(len(bs) * cfg["S"], cfg["D"]).T.astype(BF16)
        )
        e4 = e[bs][:, ::s4, :]  # [nb, S//s4, D]
        eT4c = np.ascontiguousarray(
            e4.reshape(len(bs) * (cfg["S"] // s4), cfg["D"]).T.astype(BF16)
        )
        bias = np.stack([bq[rows], bk[rows], bv[rows]]).reshape(6, 128, 1)
        in_maps.append(
            {
                "xT": xTc,
                "eT": eTc,
                "eT4": eT4c,
                "wqT": np.ascontiguousarray(Wq[rows].T.astype(BF16)),
                "wkT": np.ascontiguousarray(Wk[rows].T.astype(BF16)),
                "wvT": np.ascontiguousarray(Wv[rows].T.astype(BF16)),
                "woT": np.ascontiguousarray(Wo[:, rows].T.astype(BF16)),
                "biases": np.ascontiguousarray(bias),
            }
        )
    return in_maps


def gather_output(results, bo, cfg):
    Tl, Dl = cfg["T"], cfg["D"]
    out = np.zeros((B, Tl, Dl), np.float32)
    for c, (bs, _hs) in _core_map().items():
        p = results[c]["partial"].reshape(len(bs), Tl, Dl)
        for i, b in enumerate(bs):
            out[b] += p[i]
    return out + np.asarray(bo, np.float32)[None, None, :]


_COMPILED = None


def _get_compiled():
    global _COMPILED
    if _COMPILED is None:
        _COMPILED = build_program(FULL_CFG)
    return _COMPILED


def run_on_cores(inputs, trace=False, **kw):
    nc = _get_compiled()
    in_maps = shard_inputs(inputs, FULL_CFG)
    res = bass_utils.run_bass_kernel_spmd(
        nc, in_maps, core_ids=list(range(N_CORES)), trace=trace, **kw
    )
    return res


def kernel(**inputs) -> np.ndarray:
    res = run_on_cores(inputs, trace=False)
    return gather_output(res.results, inputs["bo"], FULL_CFG)
